# revision 1
# baseline (speedup 1.0000x reference)
"""Trainium2 Bass kernel for nn_Decoder_MLP: Linear->BN->LIF -> Linear->BN->LIF.

Sharding: data-parallel over batch B (TB=T*B=128 rows -> 4 batch items/core,
all T=4 timesteps local). BN batch stats are all-reduced across the 8 cores.

Reference semantics replicated exactly, including the "scrambled" reshapes
(T,B,N,H)->(TB,H,N) which reinterpret (N,H) blocks as (H,N) row-major. That
scramble is handled by writing LIF-1 spikes to DRAM in (m, h) row-major order
(m = r*196+n) and reading them back with a strided access pattern as
(i, m') tiles, where i = the scrambled contraction index.

Layer-1 matmul runs in native fp32 on the PE (4 cyc/row) for precision near
the LIF threshold; layer-2 uses exact-bf16 spikes with w2 split into
bf16 hi+lo parts (two matmuls, ~fp32-quality product precision).
"""

import numpy as np
import ml_dtypes

import concourse.bass as bass
import concourse.mybir as mybir
import concourse.tile as tile
from concourse import bacc
from concourse.bass_utils import run_bass_kernel_spmd
from concourse.masks import make_identity

F32 = mybir.dt.float32
BF16 = mybir.dt.bfloat16
ALU = mybir.AluOpType
ACTF = mybir.ActivationFunctionType

N_CORES = 8
T = 4
B_GLOB = 32
B_LOC = B_GLOB // N_CORES          # 4 batch items per core
R = T * B_LOC                      # 16 local (t, b) rows
NN = 196                           # sequence/pixels dim N
C = 512
H = 2048
M = R * NN                         # 3136 local rows of the flattened GEMM
M_T = 392                          # = 2*NN, keeps m-tiles r-aligned
N_MT = M // M_T                    # 8
MB = 112                           # transpose block (3136 = 28*112)
N_MB = M // MB                     # 28
C_CHUNKS = C // 128                # 4
H_TILES = H // 128                 # 16
CT_TILES = C // 128                # 4
EPS = 1e-5
NTOT = float(B_GLOB * T // T * NN * T * B_GLOB // B_GLOB)  # placeholder, unused


def rne_keep(x, bits):
    """Round-to-nearest-even keeping `bits` explicit fp32 mantissa bits —
    bit-exact emulation of the PE's fp32r input rounding (measured RNE-11)."""
    u = np.ascontiguousarray(x, dtype=np.float32).view(np.uint32)
    shift = 23 - bits
    half = np.uint32(1 << (shift - 1))
    lsb = np.uint32(1 << shift)
    mask = np.uint32(~(lsb - np.uint32(1)))
    out = (u + half - np.uint32(1) + ((u >> np.uint32(shift)) & np.uint32(1))) & mask
    return out.view(np.float32)


def _emit_lif(nc, sb_h, sp_out, vpool, s_vecs, c_vecs, n_free, n_t=T):
    """BN-apply + LIF on a (channel 128, m) tile; m = (t, b, n) t-major.
    V_t = V'_{t-1} + 2^{t-1}*(scale*h_t + shift); spike iff V_t >= 2^t;
    V'_t = V_t * (V_t < 2^t)."""
    vprev = None
    for t in range(n_t):
        thr = float(2.0 ** (t + 1))
        hsl = sb_h[t] if isinstance(sb_h, list) else \
            sb_h[:, t * n_free:(t + 1) * n_free]
        if t == 0:
            v = vpool.tile([128, n_free], F32, tag="v")
            nc.scalar.activation(out=v, in_=hsl, func=ACTF.Identity,
                                 bias=c_vecs[t], scale=s_vecs[t])
        else:
            y = vpool.tile([128, n_free], F32, tag="y")
            nc.scalar.activation(out=y, in_=hsl, func=ACTF.Identity,
                                 bias=c_vecs[t], scale=s_vecs[t])
            v = vpool.tile([128, n_free], F32, tag="v")
            nc.vector.tensor_tensor(out=v, in0=vprev, in1=y, op=ALU.add)
        nc.gpsimd.tensor_scalar(out=sp_out[:, t * n_free:(t + 1) * n_free],
                                in0=v, scalar1=thr, scalar2=None, op0=ALU.is_ge)
        if t < n_t - 1:
            vp = vpool.tile([128, n_free], F32, tag="vp")
            nc.vector.scalar_tensor_tensor(out=vp, in0=v, scalar=thr, in1=v,
                                           op0=ALU.is_lt, op1=ALU.mult)
            vprev = vp


def _emit_stats_to_scales(nc, pool, ar_sb, gamma_sb, beta_sb, w, sfx):
    """ar_sb: (128, 2w) all-reduced [sum-of-means | sum-of-Ex2]. Returns per-t
    (s_vecs, c_vecs) lists of (128, w) tiles: 2^t*scale, 2^t*shift."""
    mean = pool.tile([128, w], F32, tag=f"bnmean{sfx}", name=f"bnmean{sfx}")
    ex2 = pool.tile([128, w], F32, tag=f"bnex2{sfx}", name=f"bnex2{sfx}")
    nc.vector.tensor_scalar(out=mean, in0=ar_sb[:, 0:w], scalar1=1.0 / N_CORES,
                            scalar2=None, op0=ALU.mult)
    nc.vector.tensor_scalar(out=ex2, in0=ar_sb[:, w:2 * w],
                            scalar1=1.0 / N_CORES, scalar2=None, op0=ALU.mult)
    var = pool.tile([128, w], F32, tag=f"bnvar{sfx}", name=f"bnvar{sfx}")
    msq = pool.tile([128, w], F32, tag=f"bnmsq{sfx}", name=f"bnmsq{sfx}")
    nc.vector.tensor_tensor(out=msq, in0=mean, in1=mean, op=ALU.mult)
    nc.vector.tensor_tensor(out=var, in0=ex2, in1=msq, op=ALU.subtract)
    epsb = pool.tile([128, 1], F32, tag=f"bneps{sfx}", name=f"bneps{sfx}")
    nc.vector.memset(epsb, EPS)
    std = pool.tile([128, w], F32, tag=f"bnstd{sfx}", name=f"bnstd{sfx}")
    nc.scalar.activation(out=std, in_=var, func=ACTF.Sqrt, bias=epsb, scale=1.0)
    rstd = pool.tile([128, w], F32, tag=f"bnrstd{sfx}", name=f"bnrstd{sfx}")
    nc.vector.reciprocal(out=rstd, in_=std)
    scale = pool.tile([128, w], F32, tag=f"bnscale{sfx}", name=f"bnscale{sfx}")
    nc.vector.tensor_tensor(out=scale, in0=gamma_sb, in1=rstd, op=ALU.mult)
    mscl = pool.tile([128, w], F32, tag=f"bnmscl{sfx}", name=f"bnmscl{sfx}")
    nc.vector.tensor_tensor(out=mscl, in0=mean, in1=scale, op=ALU.mult)
    shift = pool.tile([128, w], F32, tag=f"bnshift{sfx}", name=f"bnshift{sfx}")
    nc.vector.tensor_tensor(out=shift, in0=beta_sb, in1=mscl, op=ALU.subtract)
    s_vecs, c_vecs = [], []
    for t in range(T):
        f = float(2.0 ** t)
        s = pool.tile([128, w], F32, tag=f"bns{t}{sfx}", name=f"bns{t}{sfx}")
        cc = pool.tile([128, w], F32, tag=f"bnc{t}{sfx}", name=f"bnc{t}{sfx}")
        nc.vector.tensor_scalar(out=s, in0=scale, scalar1=f, scalar2=None,
                                op0=ALU.mult)
        nc.vector.tensor_scalar(out=cc, in0=shift, scalar1=f, scalar2=None,
                                op0=ALU.mult)
        s_vecs.append(s)
        c_vecs.append(cc)
    return s_vecs, c_vecs


def build_program(stop_after='D'):
    nc = bacc.Bacc("TRN2", target_bir_lowering=False, debug=False,
                   num_devices=N_CORES)

    xTr = nc.dram_tensor("xTr", [C, M], F32, kind="ExternalInput").ap()
    xTe = nc.dram_tensor("xTe", [C, M], BF16, kind="ExternalInput").ap()
    xTh = nc.dram_tensor("xTh", [C, M], BF16, kind="ExternalInput").ap()
    w1Tr = nc.dram_tensor("w1Tr", [C, H], F32, kind="ExternalInput").ap()
    w1Te = nc.dram_tensor("w1Te", [C, H], BF16, kind="ExternalInput").ap()
    w1Th = nc.dram_tensor("w1Th", [C, H], BF16, kind="ExternalInput").ap()
    w2Thi = nc.dram_tensor("w2Thi", [H, C], BF16, kind="ExternalInput").ap()
    w2Tlo = nc.dram_tensor("w2Tlo", [H, C], BF16, kind="ExternalInput").ap()
    g1 = nc.dram_tensor("g1", [H], F32, kind="ExternalInput").ap()
    b1 = nc.dram_tensor("b1", [H], F32, kind="ExternalInput").ap()
    g2 = nc.dram_tensor("g2", [C], F32, kind="ExternalInput").ap()
    b2 = nc.dram_tensor("b2", [C], F32, kind="ExternalInput").ap()
    qv = nc.dram_tensor("qv", [M, C], F32, kind="ExternalOutput").ap()
    chain = nc.dram_tensor("chain", [1, 128], F32, kind="ExternalInput").ap()
    chain_o = nc.dram_tensor("chain_o", [1, 128], F32, kind="ExternalOutput").ap()

    with tile.TileContext(nc) as tc:
        from contextlib import ExitStack
        with ExitStack() as ctx:
            _build_body(nc, tc, ctx, (xTr, xTe, xTh), (w1Tr, w1Te, w1Th),
                        w2Thi, w2Tlo, g1, b1, g2, b2, qv, stop_after)
        with tc.tile_pool(name="chainp", bufs=1) as chp:
            cht = chp.tile([1, 128], F32)
            nc.sync.dma_start(out=cht, in_=chain)
            nc.sync.dma_start(out=chain_o, in_=cht)
    nc.compile()
    return nc


def _build_body(nc, tc, ctx, x_in, w1_in, w2Thi, w2Tlo, g1, b1, g2, b2, qv,
                stop_after='D'):
    xTr, xTe, xTh = x_in
    w1Tr, w1Te, w1Th = w1_in
    F32R = mybir.dt.float32r
    HH = H_TILES // 2            # 8 h-tiles per half
    HCOL = HH * 128              # 1024 h columns per half

    persist = ctx.enter_context(tc.tile_pool(name="persist", bufs=1))
    dram = ctx.enter_context(tc.tile_pool(name="dram", bufs=1, space="DRAM"))

    id_bf = persist.tile([128, 128], BF16)
    make_identity(nc, id_bf)

    g1_sb = persist.tile([128, H_TILES], F32)
    b1_sb = persist.tile([128, H_TILES], F32)
    g2_sb = persist.tile([128, CT_TILES], F32)
    b2_sb = persist.tile([128, CT_TILES], F32)
    nc.sync.dma_start(out=g1_sb, in_=g1.rearrange("(a b) -> b a", b=128))
    nc.sync.dma_start(out=b1_sb, in_=b1.rearrange("(a b) -> b a", b=128))
    nc.sync.dma_start(out=g2_sb, in_=g2.rearrange("(a b) -> b a", b=128))
    nc.sync.dma_start(out=b2_sb, in_=b2.rearrange("(a b) -> b a", b=128))

    h_dram = dram.tile([H_TILES, 128, M], F32)
    spk_dram = dram.tile([M, H], BF16)
    ar1_in = dram.tile([2, 128, 2 * HH], F32)
    ar1_outs = [dram.tile([128, 2 * HH], F32, addr_space="Shared",
                          tag=f"ar1o{i}", name=f"ar1o{i}") for i in range(2)]
    ar2_in = dram.tile([2, 128, 4], F32)
    ar2_outs = [dram.tile([128, 4], F32, addr_space="Shared",
                          tag=f"ar2o{i}", name=f"ar2o{i}") for i in range(2)]

    # ================= A+B superphase: two ht-halves, interleaved =========
    with tc.tile_pool(name="pA_w", bufs=2) as paw, \
         tc.tile_pool(name="pA_wf", bufs=1) as pawf, \
         tc.tile_pool(name="pA_x", bufs=2) as pax, \
         tc.tile_pool(name="pA_h", bufs=4) as pah, \
         tc.tile_pool(name="pA_st", bufs=1) as past, \
         tc.tile_pool(name="pA_ps", bufs=5, space="PSUM") as paps, \
         tc.tile_pool(name="pB_h", bufs=10) as pbh, \
         tc.tile_pool(name="pB_v", bufs=3) as pbv, \
         tc.tile_pool(name="pB_sp", bufs=1) as pbsp, \
         tc.tile_pool(name="pB_stg", bufs=4) as pbstg, \
         tc.tile_pool(name="pB_ps", bufs=2, space="PSUM") as pbps:

        w1sbs = {}
        scales1 = {}
        sp_tiles = {}

        def emit_w1_loads(half):
            csl = slice(half * HCOL, (half + 1) * HCOL)
            w1r_sb, w1e_sb, w1h_sb = [], [], []
            for c in range(C_CHUNKS):
                wf = pawf.tile([128, HCOL], F32, tag="wf", name=f"wf{half}_{c}")
                nc.sync.dma_start(out=wf, in_=w1Tr[c * 128:(c + 1) * 128, csl])
                wr = paw.tile([128, HCOL], F32R, tag=f"w1r{c}",
                              name=f"w1r{half}_{c}")
                nc.vector.tensor_copy(wr, wf)
                w1r_sb.append(wr)
                we = paw.tile([128, HCOL], BF16, tag=f"w1e{c}",
                              name=f"w1e{half}_{c}")
                wh = paw.tile([128, HCOL], BF16, tag=f"w1h{c}",
                              name=f"w1h{half}_{c}")
                nc.sync.dma_start(out=we, in_=w1Te[c * 128:(c + 1) * 128, csl])
                nc.sync.dma_start(out=wh, in_=w1Th[c * 128:(c + 1) * 128, csl])
                w1e_sb.append(we)
                w1h_sb.append(wh)
            w1sbs[half] = (w1r_sb, w1e_sb, w1h_sb)

        st_tiles = {}

        def emit_A_mt(half, mt):
            w1r_sb, w1e_sb, w1h_sb = w1sbs[half]
            hts = list(range(half * HH, (half + 1) * HH))
            if mt == 0:
                st_tiles[half] = [past.tile([128, N_MT, 6], F32,
                                            tag=f"st{ht}", name=f"st{ht}")
                                  for ht in hts]
            st = st_tiles[half]
            msl = slice(mt * M_T, (mt + 1) * M_T)
            xr_sb, xe_sb, xh_sb = [], [], []
            for c in range(C_CHUNKS):
                xf = pax.tile([128, M_T], F32, tag=f"xf{c}",
                              name=f"xf{half}_{mt}_{c}")
                nc.sync.dma_start(out=xf, in_=xTr[c * 128:(c + 1) * 128, msl])
                xr = pax.tile([128, M_T], F32R, tag=f"xr{c}",
                              name=f"xr{half}_{mt}_{c}")
                nc.vector.tensor_copy(xr, xf)
                xr_sb.append(xr)
                xe = pax.tile([128, M_T], BF16, tag=f"xe{c}",
                              name=f"xe{half}_{mt}_{c}")
                xh = pax.tile([128, M_T], BF16, tag=f"xh{c}",
                              name=f"xh{half}_{mt}_{c}")
                nc.sync.dma_start(out=xe, in_=xTe[c * 128:(c + 1) * 128, msl])
                nc.sync.dma_start(out=xh, in_=xTh[c * 128:(c + 1) * 128, msl])
                xe_sb.append(xe)
                xh_sb.append(xh)
            for i_ht, ht in enumerate(hts):
                hsl = slice(i_ht * 128, (i_ht + 1) * 128)
                ps = paps.tile([128, M_T], F32, tag="ps")
                for c in range(C_CHUNKS):
                    nc.tensor.matmul(ps, w1r_sb[c][:, hsl], xr_sb[c],
                                     start=(c == 0), stop=False)
                for c in range(C_CHUNKS):
                    nc.tensor.matmul(ps, w1h_sb[c][:, hsl], xe_sb[c],
                                     start=False, stop=False)
                for c in range(C_CHUNKS):
                    nc.tensor.matmul(ps, w1e_sb[c][:, hsl], xh_sb[c],
                                     start=False, stop=(c == C_CHUNKS - 1))
                nc.vector.bn_stats(out=st[i_ht][:, mt, :], in_=ps)
                hstg = pah.tile([128, M_T], F32, tag="hstg")
                nc.scalar.activation(out=hstg, in_=ps, func=ACTF.Copy)
                nc.sync.dma_start(out=h_dram[ht][:, msl], in_=hstg)

        def emit_stats_ar1(half):
            hts = list(range(half * HH, (half + 1) * HH))
            st = st_tiles[half]
            stats1h = persist.tile([128, 2 * HH], F32, tag=f"stats1h{half}",
                                   name=f"stats1h{half}")
            for i_ht, ht in enumerate(hts):
                mv = past.tile([128, 2], F32, tag="mv", name=f"mv{ht}")
                nc.vector.bn_aggr(out=mv, in_=st[i_ht])
                nc.vector.tensor_copy(stats1h[:, i_ht:i_ht + 1], mv[:, 0:1])
                nc.vector.scalar_tensor_tensor(
                    out=stats1h[:, HH + i_ht:HH + i_ht + 1],
                    in0=mv[:, 0:1], scalar=mv[:, 0:1], in1=mv[:, 1:2],
                    op0=ALU.mult, op1=ALU.add)
            nc.sync.dma_start(out=ar1_in[half], in_=stats1h)
            nc.gpsimd.collective_compute(
                "AllReduce", ALU.add, replica_groups=[list(range(N_CORES))],
                ins=[ar1_in[half].opt()], outs=[ar1_outs[half].opt()])
            ar1h = persist.tile([128, 2 * HH], F32, tag=f"ar1h{half}",
                                name=f"ar1h{half}")
            nc.sync.dma_start(out=ar1h, in_=ar1_outs[half])
            scales1[half] = _emit_stats_to_scales(
                nc, persist, ar1h, g1_sb[:, half * HH:(half + 1) * HH],
                b1_sb[:, half * HH:(half + 1) * HH], HH, f"L1h{half}")

        def emit_B_tile(half, i_ht):
            s1v, c1v = scales1[half]
            ht = half * HH + i_ht
            hbs = []
            for tt in range(T):
                hbt = pbh.tile([128, NN * B_LOC], F32, tag="hb",
                               name=f"hb{ht}_{tt}")
                nc.sync.dma_start(
                    out=hbt,
                    in_=h_dram[ht][:, tt * NN * B_LOC:(tt + 1) * NN * B_LOC])
                hbs.append(hbt)
            sp = pbsp.tile([128, M], BF16, tag=f"sp{i_ht % 4}",
                           name=f"sp{half}_{i_ht}")
            _emit_lif(nc, hbs, sp, pbv,
                      [s[:, i_ht:i_ht + 1] for s in s1v],
                      [cv[:, i_ht:i_ht + 1] for cv in c1v],
                      NN * B_LOC)
            sp_tiles[(half, i_ht)] = sp

        def emit_B_trans(half, hg):
            sps = [sp_tiles[(half, hg * 4 + hh)] for hh in range(4)]
            hcol0 = (half * 2 + hg) * 512
            for mb in range(N_MB):
                pst = pbps.tile([MB, 512], BF16, tag="pst")
                for hh in range(4):
                    nc.tensor.matmul(
                        pst[:, hh * 128:(hh + 1) * 128],
                        sps[hh][:, mb * MB:(mb + 1) * MB],
                        id_bf, is_transpose=True,
                        start=(hh == 0), stop=(hh == 3))
                stg = pbstg.tile([MB, 512], BF16, tag="stg")
                if mb % 3 != 2:
                    nc.scalar.activation(out=stg, in_=pst, func=ACTF.Copy)
                else:
                    nc.vector.tensor_copy(stg, pst)
                nc.sync.dma_start(
                    out=spk_dram[mb * MB:(mb + 1) * MB, hcol0:hcol0 + 512],
                    in_=stg)

        # --- interleaved A/B emission ---
        emit_w1_loads(0)
        emit_w1_loads(1)
        for mt in range(N_MT):
            emit_A_mt(0, mt)
        emit_stats_ar1(0)
        for mt in range(N_MT):
            emit_A_mt(1, mt)
            emit_B_tile(0, mt)
            if mt == 3:
                emit_B_trans(0, 0)
        emit_stats_ar1(1)
        emit_B_trans(0, 1)
        for i_ht in range(HH):
            emit_B_tile(1, i_ht)
            if i_ht == 4:
                emit_B_trans(1, 0)
        emit_B_trans(1, 1)

    if stop_after in ('A', 'B'):
        return

    # ================= C+D superphase: two ct-halves, interleaved =========
    with tc.tile_pool(name="pC_w", bufs=1) as pcw, \
         tc.tile_pool(name="pC_r", bufs=2) as pcr, \
         tc.tile_pool(name="pC_st", bufs=1) as pcst, \
         tc.tile_pool(name="pC_ps", bufs=5, space="PSUM") as pcps, \
         tc.tile_pool(name="pO", bufs=1) as po, \
         tc.tile_pool(name="pD_v", bufs=2) as pdv, \
         tc.tile_pool(name="pD_sp", bufs=1) as pdsp, \
         tc.tile_pool(name="pD_stg", bufs=4) as pdstg, \
         tc.tile_pool(name="pD_ps", bufs=2, space="PSUM") as pdps:
        w2hi_sb = [pcw.tile([128, C], BF16, tag=f"w2h{i}", name=f"w2h{i}")
                   for i in range(H_TILES)]
        w2lo_sb = [pcw.tile([128, C], BF16, tag=f"w2l{i}", name=f"w2l{i}")
                   for i in range(H_TILES)]
        for i in range(H_TILES):
            nc.sync.dma_start(out=w2hi_sb[i], in_=w2Thi[i * 128:(i + 1) * 128, :])
            nc.sync.dma_start(out=w2lo_sb[i], in_=w2Tlo[i * 128:(i + 1) * 128, :])

        sp2 = [pdsp.tile([128, M], BF16, tag=f"sp2_{ct}", name=f"sp2_{ct}")
               for ct in range(CT_TILES)]
        o_tiles = {}
        st2_tiles = {}
        scales2 = {}

        def emit_C_mt(chalf, mt):
            cts = [2 * chalf, 2 * chalf + 1]
            if mt == 0:
                o_tiles[chalf] = [po.tile([128, M], F32, tag=f"osb{ct}",
                                          name=f"osb{ct}") for ct in cts]
                st2_tiles[chalf] = [pcst.tile([128, N_MT, 6], F32,
                                              tag=f"st2_{ct}",
                                              name=f"st2_{ct}") for ct in cts]
            o_sb = o_tiles[chalf]
            st2 = st2_tiles[chalf]
            rhs = []
            for ic in range(H_TILES):
                rt = pcr.tile([128, 2, NN], BF16, tag=f"rhs{ic}",
                              name=f"rhs{chalf}_{mt}_{ic}")
                src = bass.AP(
                    tensor=spk_dram.tensor,
                    offset=spk_dram.offset + (2 * mt) * (NN * H)
                    + (ic * 128) * NN,
                    ap=[[NN, 128], [NN * H, 2], [1, NN]])
                nc.sync.dma_start(out=rt, in_=src)
                rhs.append(rt)
            for i_ct, ct in enumerate(cts):
                ps = pcps.tile([128, M_T], F32, tag="ps2")
                for ic in range(H_TILES):
                    nc.tensor.matmul(
                        ps, w2hi_sb[ic][:, ct * 128:(ct + 1) * 128],
                        rhs[ic].rearrange("p a b -> p (a b)"),
                        start=(ic == 0), stop=False)
                for ic in range(H_TILES):
                    nc.tensor.matmul(
                        ps, w2lo_sb[ic][:, ct * 128:(ct + 1) * 128],
                        rhs[ic].rearrange("p a b -> p (a b)"),
                        start=False, stop=(ic == H_TILES - 1))
                nc.vector.bn_stats(out=st2[i_ct][:, mt, :], in_=ps)
                nc.scalar.activation(
                    out=o_sb[i_ct][:, mt * M_T:(mt + 1) * M_T],
                    in_=ps, func=ACTF.Copy)

        def emit_stats_ar2(chalf):
            cts = [2 * chalf, 2 * chalf + 1]
            st2 = st2_tiles[chalf]
            stats2h = persist.tile([128, 4], F32, tag=f"stats2h{chalf}",
                                   name=f"stats2h{chalf}")
            for i_ct, ct in enumerate(cts):
                mv2 = pcst.tile([128, 2], F32, tag="mv2", name=f"mv2{ct}")
                nc.vector.bn_aggr(out=mv2, in_=st2[i_ct])
                nc.vector.tensor_copy(stats2h[:, i_ct:i_ct + 1], mv2[:, 0:1])
                nc.vector.scalar_tensor_tensor(
                    out=stats2h[:, 2 + i_ct:2 + i_ct + 1],
                    in0=mv2[:, 0:1], scalar=mv2[:, 0:1], in1=mv2[:, 1:2],
                    op0=ALU.mult, op1=ALU.add)
            nc.sync.dma_start(out=ar2_in[chalf], in_=stats2h)
            nc.gpsimd.collective_compute(
                "AllReduce", ALU.add, replica_groups=[list(range(N_CORES))],
                ins=[ar2_in[chalf].opt()], outs=[ar2_outs[chalf].opt()])
            ar2h = persist.tile([128, 4], F32, tag=f"ar2h{chalf}",
                                name=f"ar2h{chalf}")
            nc.sync.dma_start(out=ar2h, in_=ar2_outs[chalf])
            scales2[chalf] = _emit_stats_to_scales(
                nc, persist, ar2h, g2_sb[:, 2 * chalf:2 * chalf + 2],
                b2_sb[:, 2 * chalf:2 * chalf + 2], 2, f"L2h{chalf}")

        def emit_D_lif(chalf, i_ct):
            s2v, c2v = scales2[chalf]
            ct = 2 * chalf + i_ct
            _emit_lif(nc, o_tiles[chalf][i_ct], sp2[ct], pdv,
                      [s[:, i_ct:i_ct + 1] for s in s2v],
                      [cv[:, i_ct:i_ct + 1] for cv in c2v],
                      NN * B_LOC)

        def emit_D_trans(chalf):
            cts = [2 * chalf, 2 * chalf + 1]
            for mb in range(N_MB):
                pst = pdps.tile([MB, 256], BF16, tag="pst2")
                for i_ct, ct in enumerate(cts):
                    nc.tensor.matmul(
                        pst[:, i_ct * 128:(i_ct + 1) * 128],
                        sp2[ct][:, mb * MB:(mb + 1) * MB],
                        id_bf, is_transpose=True,
                        start=(i_ct == 0), stop=(i_ct == 1))
                stg = pdstg.tile([MB, 256], F32, tag="stg2")
                if mb % 2 == 0:
                    nc.scalar.activation(out=stg, in_=pst, func=ACTF.Copy)
                else:
                    nc.vector.tensor_copy(stg, pst)
                nc.sync.dma_start(
                    out=qv[mb * MB:(mb + 1) * MB,
                           chalf * 256:(chalf + 1) * 256],
                    in_=stg)

        # --- interleaved C/D emission ---
        for mt in range(N_MT):
            emit_C_mt(0, mt)
        emit_stats_ar2(0)
        for mt in range(N_MT):
            emit_C_mt(1, mt)
            if mt == 1:
                emit_D_lif(0, 0)
            elif mt == 3:
                emit_D_lif(0, 1)
        emit_stats_ar2(1)
        emit_D_trans(0)
        emit_D_lif(1, 0)
        emit_D_lif(1, 1)
        emit_D_trans(1)


_NC_CACHE = None
LAST_RES = None


def _get_nc():
    global _NC_CACHE
    if _NC_CACHE is None:
        _NC_CACHE = build_program()
    return _NC_CACHE


def kernel(x, w1, g1, b1, w2, g2, b2):
    x = np.asarray(x, dtype=np.float32)
    w1 = np.asarray(w1, dtype=np.float32)
    w2 = np.asarray(w2, dtype=np.float32)
    g1 = np.asarray(g1, dtype=np.float32)
    b1 = np.asarray(b1, dtype=np.float32)
    g2 = np.asarray(g2, dtype=np.float32)
    b2 = np.asarray(b2, dtype=np.float32)

    w1T = np.ascontiguousarray(w1.T)                    # (C, H)
    w2T = np.ascontiguousarray(w2.T).astype(np.float32)  # (H, C)
    w2Thi = w2T.astype(ml_dtypes.bfloat16)
    w2Tlo = (w2T - w2Thi.astype(np.float32)).astype(ml_dtypes.bfloat16)
    w1Tr = rne_keep(w1T, 11)
    w1Te = (w1T - w1Tr).astype(ml_dtypes.bfloat16)
    w1Th = w1T.astype(ml_dtypes.bfloat16)

    xr = x.reshape(T, B_GLOB, NN, C)
    in_maps = []
    for k in range(N_CORES):
        xk = xr[:, k * B_LOC:(k + 1) * B_LOC].reshape(M, C)
        xTk = np.ascontiguousarray(xk.T)                # (C, M)
        xTr = rne_keep(xTk, 11)
        in_maps.append({
            "xTr": xTr,
            "xTe": (xTk - xTr).astype(ml_dtypes.bfloat16),
            "xTh": xTk.astype(ml_dtypes.bfloat16),
            "w1Tr": w1Tr, "w1Te": w1Te, "w1Th": w1Th,
            "w2Thi": w2Thi, "w2Tlo": w2Tlo,
            "g1": g1, "b1": b1, "g2": g2, "b2": b2,
            "chain": np.zeros((1, 128), np.float32),
        })

    nc = _get_nc()
    import os
    trace = bool(int(os.environ.get("KERNEL_TRACE", "0")))
    res = run_bass_kernel_spmd(nc, in_maps, core_ids=list(range(N_CORES)),
                               trace=trace)
    global LAST_RES
    LAST_RES = res

    out = np.empty((T * B_GLOB, NN, C), dtype=np.float32)
    outr = out.reshape(T, B_GLOB, NN, C)
    for k in range(N_CORES):
        qvk = res.results[k]["qv"]                      # (M, C) in (r, n, c)
        # reference: reshape(TB, C, N).transpose(0, 2, 1)
        tmp = qvk.reshape(R, C, NN).transpose(0, 2, 1)  # (R, N, C)
        outr[:, k * B_LOC:(k + 1) * B_LOC] = tmp.reshape(T, B_LOC, NN, C)
    return out



# revision 5
# speedup vs baseline: 1.1165x; 1.1165x over previous
"""Trainium2 Bass kernel for nn_Decoder_MLP: Linear->BN->LIF -> Linear->BN->LIF.

Sharding: data-parallel over batch B (TB=T*B=128 rows -> 4 batch items/core,
all T=4 timesteps local). BN batch stats are all-gathered (cheaper than
all-reduce in the collective model) and summed locally on DVE.

Reference semantics replicated exactly, including the "scrambled" reshapes
(T,B,N,H)->(TB,H,N) which reinterpret (N,H) blocks as (H,N) row-major. That
scramble is handled by writing LIF-1 spikes to DRAM in (m, h) row-major order
(m = r*196+n) and reading them back with a strided access pattern as
(i, m') tiles, where i = the scrambled contraction index.

Layer-1 matmul runs fp32r (RNE-11) + two bf16 cross-term passes (~fp32
quality). Layer-2 runs on exact fp8(e4m3) spikes with w2 split into two
scale-groups of two e4m3 parts each, using DoubleRow fp8 matmuls (0.5
cyc/row); the group scales cancel in BN (scale-invariant), the inter-group
2^-8 factor is applied in the psum-combine step.
"""

import numpy as np
import ml_dtypes

import concourse.bass as bass
import concourse.mybir as mybir
import concourse.tile as tile
from concourse import bacc
from concourse.bass_utils import run_bass_kernel_spmd
from concourse.masks import make_identity

F32 = mybir.dt.float32
F32R = mybir.dt.float32r
BF16 = mybir.dt.bfloat16
FP8 = mybir.dt.float8e4
ALU = mybir.AluOpType
ACTF = mybir.ActivationFunctionType
DR = mybir.MatmulPerfMode.DoubleRow

N_CORES = 8
T = 4
B_GLOB = 32
B_LOC = B_GLOB // N_CORES          # 4 batch items per core
R = T * B_LOC                      # 16 local (t, b) rows
NN = 196                           # sequence/pixels dim N
C = 512
H = 2048
M = R * NN                         # 3136 local rows of the flattened GEMM
M_T = 392                          # = 2*NN, keeps m-tiles r-aligned
N_MT = M // M_T                    # 8
MB = 112                           # transpose block (3136 = 28*112)
N_MB = M // MB                     # 28
C_CHUNKS = C // 128                # 4
H_TILES = H // 128                 # 16
CT_TILES = C // 128                # 4
N_ICP = H // 256                   # 8 DoubleRow contraction chunks
EPS = 1e-5
GSH = 8                            # fp8 scale-group shift (2^GSH)


def rne_keep(x, bits):
    """Round-to-nearest-even keeping `bits` explicit fp32 mantissa bits —
    bit-exact emulation of the PE's fp32r input rounding (measured RNE-11)."""
    u = np.ascontiguousarray(x, dtype=np.float32).view(np.uint32)
    shift = 23 - bits
    half = np.uint32(1 << (shift - 1))
    lsb = np.uint32(1 << shift)
    mask = np.uint32(~(lsb - np.uint32(1)))
    out = (u + half - np.uint32(1) + ((u >> np.uint32(shift)) & np.uint32(1))) & mask
    return out.view(np.float32)


def _emit_lif(nc, sb_h, sp_out, vpool, s_vecs, c_vecs, n_free, n_t=T):
    """BN-apply + LIF on a (channel 128, m) tile; m = (t, b, n) t-major.
    V_t = V'_{t-1} + 2^{t-1}*(scale*h_t + shift); spike iff V_t >= 2^t;
    V'_t = V_t * (V_t < 2^t)."""
    vprev = None
    for t in range(n_t):
        thr = float(2.0 ** (t + 1))
        hsl = sb_h[t] if isinstance(sb_h, list) else \
            sb_h[:, t * n_free:(t + 1) * n_free]
        if t == 0:
            v = vpool.tile([128, n_free], F32, tag="v")
            nc.scalar.activation(out=v, in_=hsl, func=ACTF.Identity,
                                 bias=c_vecs[t], scale=s_vecs[t])
        else:
            y = vpool.tile([128, n_free], F32, tag="y")
            nc.scalar.activation(out=y, in_=hsl, func=ACTF.Identity,
                                 bias=c_vecs[t], scale=s_vecs[t])
            v = vpool.tile([128, n_free], F32, tag="v")
            nc.vector.tensor_tensor(out=v, in0=vprev, in1=y, op=ALU.add)
        nc.gpsimd.tensor_scalar(out=sp_out[:, t * n_free:(t + 1) * n_free],
                                in0=v, scalar1=thr, scalar2=None, op0=ALU.is_ge)
        if t < n_t - 1:
            vp = vpool.tile([128, n_free], F32, tag="vp")
            nc.vector.scalar_tensor_tensor(out=vp, in0=v, scalar=thr, in1=v,
                                           op0=ALU.is_lt, op1=ALU.mult)
            vprev = vp


def _emit_ag_sum(nc, pool, ag_in, ag_out, stats_sb, w, sfx):
    """AllGather per-core stats ([128, 2w]) then 8-way local sum on DVE.
    Returns the summed [128, 2w] SBUF tile."""
    nc.sync.dma_start(out=ag_in, in_=stats_sb)
    nc.gpsimd.collective_compute(
        "AllGather", ALU.bypass, replica_groups=[list(range(N_CORES))],
        ins=[ag_in.opt()], outs=[ag_out.opt()])
    agg = pool.tile([128, 8, 2 * w], F32, tag=f"agg{sfx}", name=f"agg{sfx}")
    nc.sync.dma_start(out=agg, in_=ag_out.rearrange("a p w -> p a w"))
    t4 = pool.tile([128, 4, 2 * w], F32, tag=f"t4{sfx}", name=f"t4{sfx}")
    nc.vector.tensor_tensor(out=t4, in0=agg[:, 0:4], in1=agg[:, 4:8],
                            op=ALU.add)
    t2 = pool.tile([128, 2, 2 * w], F32, tag=f"t2{sfx}", name=f"t2{sfx}")
    nc.vector.tensor_tensor(out=t2, in0=t4[:, 0:2], in1=t4[:, 2:4],
                            op=ALU.add)
    t1 = pool.tile([128, 2 * w], F32, tag=f"t1{sfx}", name=f"t1{sfx}")
    nc.vector.tensor_tensor(out=t1, in0=t2[:, 0:1], in1=t2[:, 1:2],
                            op=ALU.add)
    return t1


def _emit_stats_to_scales(nc, pool, ar_sb, gamma_sb, beta_sb, w, sfx):
    """ar_sb: (128, 2w) summed [sum-of-means | sum-of-Ex2]. Returns per-t
    (s_vecs, c_vecs) lists of (128, w) tiles: 2^t*scale, 2^t*shift."""
    mean = pool.tile([128, w], F32, tag=f"bnmean{sfx}", name=f"bnmean{sfx}")
    ex2 = pool.tile([128, w], F32, tag=f"bnex2{sfx}", name=f"bnex2{sfx}")
    nc.vector.tensor_scalar(out=mean, in0=ar_sb[:, 0:w], scalar1=1.0 / N_CORES,
                            scalar2=None, op0=ALU.mult)
    nc.vector.tensor_scalar(out=ex2, in0=ar_sb[:, w:2 * w],
                            scalar1=1.0 / N_CORES, scalar2=None, op0=ALU.mult)
    var = pool.tile([128, w], F32, tag=f"bnvar{sfx}", name=f"bnvar{sfx}")
    msq = pool.tile([128, w], F32, tag=f"bnmsq{sfx}", name=f"bnmsq{sfx}")
    nc.vector.tensor_tensor(out=msq, in0=mean, in1=mean, op=ALU.mult)
    nc.vector.tensor_tensor(out=var, in0=ex2, in1=msq, op=ALU.subtract)
    epsb = pool.tile([128, 1], F32, tag=f"bneps{sfx}", name=f"bneps{sfx}")
    nc.vector.memset(epsb, EPS)
    std = pool.tile([128, w], F32, tag=f"bnstd{sfx}", name=f"bnstd{sfx}")
    nc.scalar.activation(out=std, in_=var, func=ACTF.Sqrt, bias=epsb, scale=1.0)
    rstd = pool.tile([128, w], F32, tag=f"bnrstd{sfx}", name=f"bnrstd{sfx}")
    nc.vector.reciprocal(out=rstd, in_=std)
    scale = pool.tile([128, w], F32, tag=f"bnscale{sfx}", name=f"bnscale{sfx}")
    nc.vector.tensor_tensor(out=scale, in0=gamma_sb, in1=rstd, op=ALU.mult)
    mscl = pool.tile([128, w], F32, tag=f"bnmscl{sfx}", name=f"bnmscl{sfx}")
    nc.vector.tensor_tensor(out=mscl, in0=mean, in1=scale, op=ALU.mult)
    shift = pool.tile([128, w], F32, tag=f"bnshift{sfx}", name=f"bnshift{sfx}")
    nc.vector.tensor_tensor(out=shift, in0=beta_sb, in1=mscl, op=ALU.subtract)
    s_vecs, c_vecs = [], []
    for t in range(T):
        f = float(2.0 ** t)
        s = pool.tile([128, w], F32, tag=f"bns{t}{sfx}", name=f"bns{t}{sfx}")
        cc = pool.tile([128, w], F32, tag=f"bnc{t}{sfx}", name=f"bnc{t}{sfx}")
        nc.vector.tensor_scalar(out=s, in0=scale, scalar1=f, scalar2=None,
                                op0=ALU.mult)
        nc.vector.tensor_scalar(out=cc, in0=shift, scalar1=f, scalar2=None,
                                op0=ALU.mult)
        s_vecs.append(s)
        c_vecs.append(cc)
    return s_vecs, c_vecs


def build_program(stop_after='D'):
    nc = bacc.Bacc("TRN2", target_bir_lowering=False, debug=False,
                   num_devices=N_CORES)

    xTr = nc.dram_tensor("xTr", [C, M], F32R, kind="ExternalInput").ap()
    xTe = nc.dram_tensor("xTe", [C, M], BF16, kind="ExternalInput").ap()
    xTh = nc.dram_tensor("xTh", [C, M], BF16, kind="ExternalInput").ap()
    w1Tr = nc.dram_tensor("w1Tr", [C, H], F32R, kind="ExternalInput").ap()
    w1Te = nc.dram_tensor("w1Te", [C, H], BF16, kind="ExternalInput").ap()
    w1Th = nc.dram_tensor("w1Th", [C, H], BF16, kind="ExternalInput").ap()
    w2p = [nc.dram_tensor(f"w2p{p}", [H, C], FP8, kind="ExternalInput").ap()
           for p in range(4)]
    g1 = nc.dram_tensor("g1", [H], F32, kind="ExternalInput").ap()
    b1 = nc.dram_tensor("b1", [H], F32, kind="ExternalInput").ap()
    g2 = nc.dram_tensor("g2", [C], F32, kind="ExternalInput").ap()
    b2 = nc.dram_tensor("b2", [C], F32, kind="ExternalInput").ap()
    qv = nc.dram_tensor("qv", [M, C], F32, kind="ExternalOutput").ap()
    chain = nc.dram_tensor("chain", [1, 128], F32, kind="ExternalInput").ap()
    chain_o = nc.dram_tensor("chain_o", [1, 128], F32, kind="ExternalOutput").ap()

    with tile.TileContext(nc) as tc:
        from contextlib import ExitStack
        with ExitStack() as ctx:
            _build_body(nc, tc, ctx, (xTr, xTe, xTh), (w1Tr, w1Te, w1Th),
                        w2p, g1, b1, g2, b2, qv, stop_after)
        with tc.tile_pool(name="chainp", bufs=1) as chp:
            cht = chp.tile([1, 128], F32)
            nc.sync.dma_start(out=cht, in_=chain)
            nc.sync.dma_start(out=chain_o, in_=cht)
    nc.compile()
    return nc


def _build_body(nc, tc, ctx, x_in, w1_in, w2p, g1, b1, g2, b2, qv,
                stop_after='D'):
    xTr, xTe, xTh = x_in
    w1Tr, w1Te, w1Th = w1_in
    HH = H_TILES // 2            # 8 h-tiles per half
    HCOL = HH * 128              # 1024 h columns per half

    persist = ctx.enter_context(tc.tile_pool(name="persist", bufs=1))
    dram = ctx.enter_context(tc.tile_pool(name="dram", bufs=1, space="DRAM"))

    id_bf = persist.tile([128, 128], BF16)
    make_identity(nc, id_bf)

    g1_sb = persist.tile([128, H_TILES], F32)
    b1_sb = persist.tile([128, H_TILES], F32)
    g2_sb = persist.tile([128, CT_TILES], F32)
    b2_sb = persist.tile([128, CT_TILES], F32)
    nc.sync.dma_start(out=g1_sb, in_=g1.rearrange("(a b) -> b a", b=128))
    nc.sync.dma_start(out=b1_sb, in_=b1.rearrange("(a b) -> b a", b=128))
    nc.sync.dma_start(out=g2_sb, in_=g2.rearrange("(a b) -> b a", b=128))
    nc.sync.dma_start(out=b2_sb, in_=b2.rearrange("(a b) -> b a", b=128))

    h_dram = dram.tile([H_TILES, 128, M], F32)
    spk_dram = dram.tile([M, H], FP8)
    ag1_in = dram.tile([2, 128, 2 * HH], F32)
    ag1_outs = [dram.tile([8, 128, 2 * HH], F32, addr_space="Shared",
                          tag=f"ag1o{i}", name=f"ag1o{i}") for i in range(2)]
    ag2_in = dram.tile([128, 8], F32)
    ag2_out = dram.tile([8, 128, 8], F32, addr_space="Shared",
                        tag="ag2o", name="ag2o")

    # ================= A+B superphase: two ht-halves, interleaved =========
    with tc.tile_pool(name="pA_w", bufs=2) as paw, \
         tc.tile_pool(name="pA_x", bufs=2) as pax, \
         tc.tile_pool(name="pA_h", bufs=4) as pah, \
         tc.tile_pool(name="pA_st", bufs=1) as past, \
         tc.tile_pool(name="pA_ps", bufs=5, space="PSUM") as paps, \
         tc.tile_pool(name="pB_h", bufs=10) as pbh, \
         tc.tile_pool(name="pB_v", bufs=3) as pbv, \
         tc.tile_pool(name="pB_sp", bufs=1) as pbsp, \
         tc.tile_pool(name="pB_stg", bufs=4) as pbstg, \
         tc.tile_pool(name="pB_ps", bufs=2, space="PSUM") as pbps:

        w1sbs = {}
        scales1 = {}
        sp_tiles = {}

        def emit_w1_loads(half):
            csl = slice(half * HCOL, (half + 1) * HCOL)
            w1r_sb, w1e_sb, w1h_sb = [], [], []
            for c in range(C_CHUNKS):
                wr = paw.tile([128, HCOL], F32R, tag=f"w1r{c}",
                              name=f"w1r{half}_{c}")
                nc.sync.dma_start(out=wr, in_=w1Tr[c * 128:(c + 1) * 128, csl])
                w1r_sb.append(wr)
                we = paw.tile([128, HCOL], BF16, tag=f"w1e{c}",
                              name=f"w1e{half}_{c}")
                wh = paw.tile([128, HCOL], BF16, tag=f"w1h{c}",
                              name=f"w1h{half}_{c}")
                nc.sync.dma_start(out=we, in_=w1Te[c * 128:(c + 1) * 128, csl])
                nc.sync.dma_start(out=wh, in_=w1Th[c * 128:(c + 1) * 128, csl])
                w1e_sb.append(we)
                w1h_sb.append(wh)
            w1sbs[half] = (w1r_sb, w1e_sb, w1h_sb)

        st_tiles = {}

        def emit_A_mt(half, mt):
            w1r_sb, w1e_sb, w1h_sb = w1sbs[half]
            hts = list(range(half * HH, (half + 1) * HH))
            if mt == 0:
                st_tiles[half] = [past.tile([128, N_MT, 6], F32,
                                            tag=f"st{ht}", name=f"st{ht}")
                                  for ht in hts]
            st = st_tiles[half]
            msl = slice(mt * M_T, (mt + 1) * M_T)
            xr_sb, xe_sb, xh_sb = [], [], []
            for c in range(C_CHUNKS):
                xr = pax.tile([128, M_T], F32R, tag=f"xr{c}",
                              name=f"xr{half}_{mt}_{c}")
                nc.sync.dma_start(out=xr, in_=xTr[c * 128:(c + 1) * 128, msl])
                xr_sb.append(xr)
                xe = pax.tile([128, M_T], BF16, tag=f"xe{c}",
                              name=f"xe{half}_{mt}_{c}")
                xh = pax.tile([128, M_T], BF16, tag=f"xh{c}",
                              name=f"xh{half}_{mt}_{c}")
                nc.sync.dma_start(out=xe, in_=xTe[c * 128:(c + 1) * 128, msl])
                nc.sync.dma_start(out=xh, in_=xTh[c * 128:(c + 1) * 128, msl])
                xe_sb.append(xe)
                xh_sb.append(xh)
            for i_ht, ht in enumerate(hts):
                hsl = slice(i_ht * 128, (i_ht + 1) * 128)
                ps = paps.tile([128, M_T], F32, tag="ps")
                for c in range(C_CHUNKS):
                    nc.tensor.matmul(ps, w1r_sb[c][:, hsl], xr_sb[c],
                                     start=(c == 0), stop=False)
                for c in range(C_CHUNKS):
                    nc.tensor.matmul(ps, w1h_sb[c][:, hsl], xe_sb[c],
                                     start=False, stop=False)
                for c in range(C_CHUNKS):
                    nc.tensor.matmul(ps, w1e_sb[c][:, hsl], xh_sb[c],
                                     start=False, stop=(c == C_CHUNKS - 1))
                nc.vector.bn_stats(out=st[i_ht][:, mt, :], in_=ps)
                hstg = pah.tile([128, M_T], F32, tag="hstg")
                nc.scalar.activation(out=hstg, in_=ps, func=ACTF.Copy)
                nc.sync.dma_start(out=h_dram[ht][:, msl], in_=hstg)

        def emit_stats_ar1(half):
            hts = list(range(half * HH, (half + 1) * HH))
            st = st_tiles[half]
            stats1h = persist.tile([128, 2 * HH], F32, tag=f"stats1h{half}",
                                   name=f"stats1h{half}")
            for i_ht, ht in enumerate(hts):
                mv = past.tile([128, 2], F32, tag="mv", name=f"mv{ht}")
                nc.vector.bn_aggr(out=mv, in_=st[i_ht])
                nc.vector.tensor_copy(stats1h[:, i_ht:i_ht + 1], mv[:, 0:1])
                nc.vector.scalar_tensor_tensor(
                    out=stats1h[:, HH + i_ht:HH + i_ht + 1],
                    in0=mv[:, 0:1], scalar=mv[:, 0:1], in1=mv[:, 1:2],
                    op0=ALU.mult, op1=ALU.add)
            ar1h = _emit_ag_sum(nc, persist, ag1_in[half], ag1_outs[half],
                                stats1h, HH, f"L1h{half}")
            scales1[half] = _emit_stats_to_scales(
                nc, persist, ar1h, g1_sb[:, half * HH:(half + 1) * HH],
                b1_sb[:, half * HH:(half + 1) * HH], HH, f"L1h{half}")

        def emit_B_tile(half, i_ht):
            s1v, c1v = scales1[half]
            ht = half * HH + i_ht
            hbs = []
            for tt in range(T):
                hbt = pbh.tile([128, NN * B_LOC], F32, tag="hb",
                               name=f"hb{ht}_{tt}")
                nc.sync.dma_start(
                    out=hbt,
                    in_=h_dram[ht][:, tt * NN * B_LOC:(tt + 1) * NN * B_LOC])
                hbs.append(hbt)
            sp = pbsp.tile([128, M], BF16, tag=f"sp{i_ht % 4}",
                           name=f"sp{half}_{i_ht}")
            _emit_lif(nc, hbs, sp, pbv,
                      [s[:, i_ht:i_ht + 1] for s in s1v],
                      [cv[:, i_ht:i_ht + 1] for cv in c1v],
                      NN * B_LOC)
            sp_tiles[(half, i_ht)] = sp

        def emit_B_trans(half, hg):
            sps = [sp_tiles[(half, hg * 4 + hh)] for hh in range(4)]
            hcol0 = (half * 2 + hg) * 512
            for mb in range(N_MB):
                pst = pbps.tile([MB, 512], BF16, tag="pst")
                for hh in range(4):
                    nc.tensor.matmul(
                        pst[:, hh * 128:(hh + 1) * 128],
                        sps[hh][:, mb * MB:(mb + 1) * MB],
                        id_bf, is_transpose=True,
                        start=(hh == 0), stop=(hh == 3))
                stg = pbstg.tile([MB, 512], FP8, tag="stg")
                if mb % 3 != 2:
                    nc.scalar.activation(out=stg, in_=pst, func=ACTF.Copy)
                else:
                    nc.vector.tensor_copy(stg, pst)
                nc.sync.dma_start(
                    out=spk_dram[mb * MB:(mb + 1) * MB, hcol0:hcol0 + 512],
                    in_=stg)

        # --- interleaved A/B emission ---
        emit_w1_loads(0)
        emit_w1_loads(1)
        for mt in range(N_MT):
            emit_A_mt(0, mt)
        emit_stats_ar1(0)
        for mt in range(N_MT):
            emit_A_mt(1, mt)
            emit_B_tile(0, mt)
            if mt == 3:
                emit_B_trans(0, 0)
        emit_stats_ar1(1)
        emit_B_trans(0, 1)
        for i_ht in range(HH):
            emit_B_tile(1, i_ht)
            if i_ht == 4:
                emit_B_trans(1, 0)
        emit_B_trans(1, 1)

    if stop_after in ('A', 'B'):
        return

    # ================= C+D: fp8 DoubleRow GEMM, all 4 ct per rhs read =====
    with tc.tile_pool(name="pC_w", bufs=1) as pcw, \
         tc.tile_pool(name="pC_r", bufs=2) as pcr, \
         tc.tile_pool(name="pC_st", bufs=1) as pcst, \
         tc.tile_pool(name="pC_ps", bufs=2, space="PSUM") as pcps, \
         tc.tile_pool(name="pO", bufs=1) as po, \
         tc.tile_pool(name="pD_v", bufs=2) as pdv, \
         tc.tile_pool(name="pD_sp", bufs=1) as pdsp, \
         tc.tile_pool(name="pD_stg", bufs=4) as pdstg, \
         tc.tile_pool(name="pD_ps", bufs=2, space="PSUM") as pdps:
        w2sb = [[pcw.tile([128, 2, C], FP8, tag=f"w2_{p}_{icp}",
                          name=f"w2_{p}_{icp}") for icp in range(N_ICP)]
                for p in range(4)]
        for p in range(4):
            for icp in range(N_ICP):
                src = bass.AP(
                    tensor=w2p[p].tensor,
                    offset=w2p[p].offset + icp * 256 * C,
                    ap=[[C, 128], [128 * C, 2], [1, C]])
                nc.sync.dma_start(out=w2sb[p][icp], in_=src)

        o_sb = [po.tile([128, M], F32, tag=f"osb{ct}", name=f"osb{ct}")
                for ct in range(CT_TILES)]
        st2 = [pcst.tile([128, N_MT, 6], F32, tag=f"st2_{ct}",
                         name=f"st2_{ct}") for ct in range(CT_TILES)]
        sp2 = [pdsp.tile([128, M], BF16, tag=f"sp2_{ct}", name=f"sp2_{ct}")
               for ct in range(CT_TILES)]

        def emit_C_mt(mt):
            rhs = []
            for icp in range(N_ICP):
                rt = pcr.tile([128, 2, 2, NN], FP8, tag=f"rhs{icp}",
                              name=f"rhs{mt}_{icp}")
                for r in range(2):
                    src = bass.AP(
                        tensor=spk_dram.tensor,
                        offset=spk_dram.offset + (2 * mt + r) * (NN * H)
                        + icp * 256 * NN,
                        ap=[[NN, 128], [128 * NN, 2], [1, NN]])
                    nc.sync.dma_start(out=rt[:, :, r, :], in_=src)
                rhs.append(rt)
            for ct in range(CT_TILES):
                csl = slice(ct * 128, (ct + 1) * 128)
                psA = pcps.tile([128, M_T], F32, tag="psA")
                psB = pcps.tile([128, M_T], F32, tag="psB")
                for g, ps in ((0, psA), (1, psB)):
                    for pp in range(2):
                        p = 2 * g + pp
                        for icp in range(N_ICP):
                            nc.tensor.matmul(
                                ps, w2sb[p][icp][:, :, csl],
                                rhs[icp][:, :, :, :],
                                perf_mode=DR,
                                start=(pp == 0 and icp == 0),
                                stop=(pp == 1 and icp == N_ICP - 1))
                # rescale to TRUE o magnitude (1/64) so BN's +eps keeps the
                # reference semantics (BN is only scale-invariant if eps
                # scales too); group-1 psum additionally carries 2^-GSH.
                osl = o_sb[ct][:, mt * M_T:(mt + 1) * M_T]
                tmpB = pcr.tile([128, M_T], F32, tag="tmpB")
                nc.scalar.activation(out=tmpB, in_=psB, func=ACTF.Identity,
                                     scale=float(2.0 ** -GSH / 64.0))
                nc.vector.scalar_tensor_tensor(
                    out=osl, in0=psA, scalar=float(1.0 / 64.0), in1=tmpB,
                    op0=ALU.mult, op1=ALU.add)
                nc.vector.bn_stats(out=st2[ct][:, mt, :], in_=osl)

        scales2 = {}

        def emit_stats_ag2():
            stats2 = persist.tile([128, 8], F32, tag="stats2", name="stats2")
            for ct in range(CT_TILES):
                mv2 = pcst.tile([128, 2], F32, tag="mv2", name=f"mv2{ct}")
                nc.vector.bn_aggr(out=mv2, in_=st2[ct])
                nc.vector.tensor_copy(stats2[:, ct:ct + 1], mv2[:, 0:1])
                nc.vector.scalar_tensor_tensor(
                    out=stats2[:, 4 + ct:5 + ct],
                    in0=mv2[:, 0:1], scalar=mv2[:, 0:1], in1=mv2[:, 1:2],
                    op0=ALU.mult, op1=ALU.add)
            ar2_sb = _emit_ag_sum(nc, persist, ag2_in, ag2_out, stats2,
                                  CT_TILES, "L2")
            scales2[0] = _emit_stats_to_scales(
                nc, persist, ar2_sb, g2_sb, b2_sb, CT_TILES, "L2")

        def emit_D_lif(ct):
            s2v, c2v = scales2[0]
            _emit_lif(nc, o_sb[ct], sp2[ct], pdv,
                      [s[:, ct:ct + 1] for s in s2v],
                      [cv[:, ct:ct + 1] for cv in c2v],
                      NN * B_LOC)

        def emit_D_trans(pair):
            cts = [2 * pair, 2 * pair + 1]
            for mb in range(N_MB):
                pst = pdps.tile([MB, 256], BF16, tag="pst2")
                for i_ct, ct in enumerate(cts):
                    nc.tensor.matmul(
                        pst[:, i_ct * 128:(i_ct + 1) * 128],
                        sp2[ct][:, mb * MB:(mb + 1) * MB],
                        id_bf, is_transpose=True,
                        start=(i_ct == 0), stop=(i_ct == 1))
                stg = pdstg.tile([MB, 256], F32, tag="stg2")
                if mb % 2 == 0:
                    nc.scalar.activation(out=stg, in_=pst, func=ACTF.Copy)
                else:
                    nc.vector.tensor_copy(stg, pst)
                nc.sync.dma_start(
                    out=qv[mb * MB:(mb + 1) * MB,
                           pair * 256:(pair + 1) * 256],
                    in_=stg)

        # --- C/D emission ---
        for mt in range(N_MT):
            emit_C_mt(mt)
        emit_stats_ag2()
        emit_D_lif(0)
        emit_D_lif(1)
        emit_D_trans(0)
        emit_D_lif(2)
        emit_D_lif(3)
        emit_D_trans(1)


_NC_CACHE = None
LAST_RES = None


def _get_nc():
    global _NC_CACHE
    if _NC_CACHE is None:
        _NC_CACHE = build_program()
    return _NC_CACHE


def kernel(x, w1, g1, b1, w2, g2, b2):
    x = np.asarray(x, dtype=np.float32)
    w1 = np.asarray(w1, dtype=np.float32)
    w2 = np.asarray(w2, dtype=np.float32)
    g1 = np.asarray(g1, dtype=np.float32)
    b1 = np.asarray(b1, dtype=np.float32)
    g2 = np.asarray(g2, dtype=np.float32)
    b2 = np.asarray(b2, dtype=np.float32)

    w1T = np.ascontiguousarray(w1.T)                    # (C, H)
    w2T = np.ascontiguousarray(w2.T).astype(np.float32)  # (H, C)
    # fp8 scale-grouped split: two groups of two e4m3 parts. Group 0 at
    # scale 2^6, group 1 at 2^(6+GSH); BN cancels the global 2^6, the
    # kernel applies 2^-GSH when combining the two group psums.
    e4 = ml_dtypes.float8_e4m3
    w2s = (w2T * 64.0).astype(np.float32)
    p0 = w2s.astype(e4)
    r1 = w2s - p0.astype(np.float32)
    p1 = r1.astype(e4)
    r2 = ((w2s - p0.astype(np.float32) - p1.astype(np.float32))
          * float(2.0 ** GSH)).astype(np.float32)
    p2 = r2.astype(e4)
    p3 = (r2 - p2.astype(np.float32)).astype(e4)
    w1Tr = rne_keep(w1T, 11)
    w1Te = (w1T - w1Tr).astype(ml_dtypes.bfloat16)
    w1Th = w1T.astype(ml_dtypes.bfloat16)

    xr = x.reshape(T, B_GLOB, NN, C)
    in_maps = []
    for k in range(N_CORES):
        xk = xr[:, k * B_LOC:(k + 1) * B_LOC].reshape(M, C)
        xTk = np.ascontiguousarray(xk.T)                # (C, M)
        xTr = rne_keep(xTk, 11)
        in_maps.append({
            "xTr": xTr,
            "xTe": (xTk - xTr).astype(ml_dtypes.bfloat16),
            "xTh": xTk.astype(ml_dtypes.bfloat16),
            "w1Tr": w1Tr, "w1Te": w1Te, "w1Th": w1Th,
            "w2p0": p0, "w2p1": p1, "w2p2": p2, "w2p3": p3,
            "g1": g1, "b1": b1, "g2": g2, "b2": b2,
            "chain": np.zeros((1, 128), np.float32),
        })

    nc = _get_nc()
    import os
    trace = bool(int(os.environ.get("KERNEL_TRACE", "0")))
    res = run_bass_kernel_spmd(nc, in_maps, core_ids=list(range(N_CORES)),
                               trace=trace)
    global LAST_RES
    LAST_RES = res

    out = np.empty((T * B_GLOB, NN, C), dtype=np.float32)
    outr = out.reshape(T, B_GLOB, NN, C)
    for k in range(N_CORES):
        qvk = res.results[k]["qv"]                      # (M, C) in (r, n, c)
        tmp = qvk.reshape(R, C, NN).transpose(0, 2, 1)  # (R, N, C)
        outr[:, k * B_LOC:(k + 1) * B_LOC] = tmp.reshape(T, B_LOC, NN, C)
    return out


# revision 11
# speedup vs baseline: 1.1841x; 1.0605x over previous
"""Trainium2 Bass kernel for nn_Decoder_MLP: Linear->BN->LIF -> Linear->BN->LIF.

Sharding: data-parallel over batch B (TB=T*B=128 rows -> 4 batch items/core,
all T=4 timesteps local). BN batch stats are all-gathered (cheaper than
all-reduce in the collective model) and summed locally on DVE.

Reference semantics replicated exactly, including the "scrambled" reshapes
(T,B,N,H)->(TB,H,N) which reinterpret (N,H) blocks as (H,N) row-major. That
scramble is handled by writing LIF-1 spikes to DRAM in (m, h) row-major order
(m = r*196+n) and reading them back with a strided access pattern as
(i, m') tiles, where i = the scrambled contraction index.

Layer-1 matmul runs fp32r (RNE-11) + two bf16 cross-term passes (~fp32
quality). Layer-2 runs on exact fp8(e4m3) spikes with w2 split into two
scale-groups of two e4m3 parts each, using DoubleRow fp8 matmuls (0.5
cyc/row); the group scales cancel in BN (scale-invariant), the inter-group
2^-8 factor is applied in the psum-combine step.
"""

import numpy as np
import ml_dtypes

import concourse.bass as bass
import concourse.mybir as mybir
import concourse.tile as tile
from concourse import bacc
from concourse.bass_utils import run_bass_kernel_spmd
from concourse.masks import make_identity

F32 = mybir.dt.float32
F32R = mybir.dt.float32r
BF16 = mybir.dt.bfloat16
FP8 = mybir.dt.float8e4
ALU = mybir.AluOpType
ACTF = mybir.ActivationFunctionType
DR = mybir.MatmulPerfMode.DoubleRow

N_CORES = 8
T = 4
B_GLOB = 32
B_LOC = B_GLOB // N_CORES          # 4 batch items per core
R = T * B_LOC                      # 16 local (t, b) rows
NN = 196                           # sequence/pixels dim N
C = 512
H = 2048
M = R * NN                         # 3136 local rows of the flattened GEMM
M_T = 392                          # = 2*NN, keeps m-tiles r-aligned
N_MT = M // M_T                    # 8
MB = 112                           # transpose block (3136 = 28*112)
N_MB = M // MB                     # 28
C_CHUNKS = C // 128                # 4
H_TILES = H // 128                 # 16
CT_TILES = C // 128                # 4
N_ICP = H // 256                   # 8 DoubleRow contraction chunks
EPS = 1e-5
GSH = 8                            # fp8 scale-group shift (2^GSH)


def rne_keep(x, bits):
    """Round-to-nearest-even keeping `bits` explicit fp32 mantissa bits —
    bit-exact emulation of the PE's fp32r input rounding (measured RNE-11)."""
    u = np.ascontiguousarray(x, dtype=np.float32).view(np.uint32)
    shift = 23 - bits
    half = np.uint32(1 << (shift - 1))
    lsb = np.uint32(1 << shift)
    mask = np.uint32(~(lsb - np.uint32(1)))
    out = (u + half - np.uint32(1) + ((u >> np.uint32(shift)) & np.uint32(1))) & mask
    return out.view(np.float32)


def _emit_lif(nc, sb_h, sp_out, vpool, s_vecs, c_vecs, n_free, n_t=T):
    """BN-apply + LIF on a (channel 128, m) tile; m = (t, b, n) t-major.
    V_t = V'_{t-1} + 2^{t-1}*(scale*h_t + shift); spike iff V_t >= 2^t;
    V'_t = V_t * (V_t < 2^t)."""
    vprev = None
    for t in range(n_t):
        thr = float(2.0 ** (t + 1))
        hsl = sb_h[t] if isinstance(sb_h, list) else \
            sb_h[:, t * n_free:(t + 1) * n_free]
        if t == 0:
            v = vpool.tile([128, n_free], F32, tag="v")
            nc.scalar.activation(out=v, in_=hsl, func=ACTF.Identity,
                                 bias=c_vecs[t], scale=s_vecs[t])
        else:
            y = vpool.tile([128, n_free], F32, tag="y")
            nc.scalar.activation(out=y, in_=hsl, func=ACTF.Identity,
                                 bias=c_vecs[t], scale=s_vecs[t])
            v = vpool.tile([128, n_free], F32, tag="v")
            nc.vector.tensor_tensor(out=v, in0=vprev, in1=y, op=ALU.add)
        nc.gpsimd.tensor_scalar(out=sp_out[:, t * n_free:(t + 1) * n_free],
                                in0=v, scalar1=thr, scalar2=None, op0=ALU.is_ge)
        if t < n_t - 1:
            vp = vpool.tile([128, n_free], F32, tag="vp")
            nc.vector.scalar_tensor_tensor(out=vp, in0=v, scalar=thr, in1=v,
                                           op0=ALU.is_lt, op1=ALU.mult)
            vprev = vp


def _emit_ag_sum(nc, pool, ag_in, ag_out, stats_sb, w, sfx):
    """AllGather per-core stats ([128, 2w]) then 8-way local sum on DVE.
    Returns the summed [128, 2w] SBUF tile."""
    nc.sync.dma_start(out=ag_in, in_=stats_sb)
    nc.gpsimd.collective_compute(
        "AllGather", ALU.bypass, replica_groups=[list(range(N_CORES))],
        ins=[ag_in.opt()], outs=[ag_out.opt()])
    agg = pool.tile([128, 8, 2 * w], F32, tag=f"agg{sfx}", name=f"agg{sfx}")
    nc.sync.dma_start(out=agg, in_=ag_out.rearrange("a p w -> p a w"))
    t4 = pool.tile([128, 4, 2 * w], F32, tag=f"t4{sfx}", name=f"t4{sfx}")
    nc.vector.tensor_tensor(out=t4, in0=agg[:, 0:4], in1=agg[:, 4:8],
                            op=ALU.add)
    t2 = pool.tile([128, 2, 2 * w], F32, tag=f"t2{sfx}", name=f"t2{sfx}")
    nc.vector.tensor_tensor(out=t2, in0=t4[:, 0:2], in1=t4[:, 2:4],
                            op=ALU.add)
    t1 = pool.tile([128, 2 * w], F32, tag=f"t1{sfx}", name=f"t1{sfx}")
    nc.vector.tensor_tensor(out=t1, in0=t2[:, 0:1], in1=t2[:, 1:2],
                            op=ALU.add)
    return t1


def _emit_stats_to_scales(nc, pool, ar_sb, gamma_sb, beta_sb, w, sfx):
    """ar_sb: (128, 2w) summed [sum-of-means | sum-of-Ex2]. Returns per-t
    (s_vecs, c_vecs) lists of (128, w) tiles: 2^t*scale, 2^t*shift."""
    mean = pool.tile([128, w], F32, tag=f"bnmean{sfx}", name=f"bnmean{sfx}")
    ex2 = pool.tile([128, w], F32, tag=f"bnex2{sfx}", name=f"bnex2{sfx}")
    nc.vector.tensor_scalar(out=mean, in0=ar_sb[:, 0:w], scalar1=1.0 / N_CORES,
                            scalar2=None, op0=ALU.mult)
    nc.vector.tensor_scalar(out=ex2, in0=ar_sb[:, w:2 * w],
                            scalar1=1.0 / N_CORES, scalar2=None, op0=ALU.mult)
    var = pool.tile([128, w], F32, tag=f"bnvar{sfx}", name=f"bnvar{sfx}")
    msq = pool.tile([128, w], F32, tag=f"bnmsq{sfx}", name=f"bnmsq{sfx}")
    nc.vector.tensor_tensor(out=msq, in0=mean, in1=mean, op=ALU.mult)
    nc.vector.tensor_tensor(out=var, in0=ex2, in1=msq, op=ALU.subtract)
    epsb = pool.tile([128, 1], F32, tag=f"bneps{sfx}", name=f"bneps{sfx}")
    nc.vector.memset(epsb, EPS)
    std = pool.tile([128, w], F32, tag=f"bnstd{sfx}", name=f"bnstd{sfx}")
    nc.scalar.activation(out=std, in_=var, func=ACTF.Sqrt, bias=epsb, scale=1.0)
    rstd = pool.tile([128, w], F32, tag=f"bnrstd{sfx}", name=f"bnrstd{sfx}")
    nc.vector.reciprocal(out=rstd, in_=std)
    scale = pool.tile([128, w], F32, tag=f"bnscale{sfx}", name=f"bnscale{sfx}")
    nc.vector.tensor_tensor(out=scale, in0=gamma_sb, in1=rstd, op=ALU.mult)
    mscl = pool.tile([128, w], F32, tag=f"bnmscl{sfx}", name=f"bnmscl{sfx}")
    nc.vector.tensor_tensor(out=mscl, in0=mean, in1=scale, op=ALU.mult)
    shift = pool.tile([128, w], F32, tag=f"bnshift{sfx}", name=f"bnshift{sfx}")
    nc.vector.tensor_tensor(out=shift, in0=beta_sb, in1=mscl, op=ALU.subtract)
    s_vecs, c_vecs = [], []
    for t in range(T):
        f = float(2.0 ** t)
        s = pool.tile([128, w], F32, tag=f"bns{t}{sfx}", name=f"bns{t}{sfx}")
        cc = pool.tile([128, w], F32, tag=f"bnc{t}{sfx}", name=f"bnc{t}{sfx}")
        nc.vector.tensor_scalar(out=s, in0=scale, scalar1=f, scalar2=None,
                                op0=ALU.mult)
        nc.vector.tensor_scalar(out=cc, in0=shift, scalar1=f, scalar2=None,
                                op0=ALU.mult)
        s_vecs.append(s)
        c_vecs.append(cc)
    return s_vecs, c_vecs


def build_program(stop_after='D'):
    nc = bacc.Bacc("TRN2", target_bir_lowering=False, debug=False,
                   num_devices=N_CORES)

    xTr = nc.dram_tensor("xTr", [C, M], F32R, kind="ExternalInput").ap()
    xTe = nc.dram_tensor("xTe", [C, M], BF16, kind="ExternalInput").ap()
    xTh = nc.dram_tensor("xTh", [C, M], BF16, kind="ExternalInput").ap()
    w1Tr = nc.dram_tensor("w1Tr", [C, H], F32R, kind="ExternalInput").ap()
    w1Te = nc.dram_tensor("w1Te", [C, H], BF16, kind="ExternalInput").ap()
    w1Th = nc.dram_tensor("w1Th", [C, H], BF16, kind="ExternalInput").ap()
    w2p = [nc.dram_tensor(f"w2p{p}", [H, C], FP8, kind="ExternalInput").ap()
           for p in range(4)]
    g1 = nc.dram_tensor("g1", [H], F32, kind="ExternalInput").ap()
    b1 = nc.dram_tensor("b1", [H], F32, kind="ExternalInput").ap()
    g2 = nc.dram_tensor("g2", [C], F32, kind="ExternalInput").ap()
    b2 = nc.dram_tensor("b2", [C], F32, kind="ExternalInput").ap()
    qv = nc.dram_tensor("qv", [M, C], F32, kind="ExternalOutput").ap()
    chain = nc.dram_tensor("chain", [1, 128], F32, kind="ExternalInput").ap()
    chain_o = nc.dram_tensor("chain_o", [1, 128], F32, kind="ExternalOutput").ap()

    with tile.TileContext(nc) as tc:
        from contextlib import ExitStack
        with ExitStack() as ctx:
            _build_body(nc, tc, ctx, (xTr, xTe, xTh), (w1Tr, w1Te, w1Th),
                        w2p, g1, b1, g2, b2, qv, stop_after)
        with tc.tile_pool(name="chainp", bufs=1) as chp:
            cht = chp.tile([1, 128], F32)
            nc.sync.dma_start(out=cht, in_=chain)
            nc.sync.dma_start(out=chain_o, in_=cht)
    nc.compile()
    return nc


def _build_body(nc, tc, ctx, x_in, w1_in, w2p, g1, b1, g2, b2, qv,
                stop_after='D'):
    xTr, xTe, xTh = x_in
    w1Tr, w1Te, w1Th = w1_in
    HH = H_TILES // 2            # 8 h-tiles per half
    HCOL = HH * 128              # 1024 h columns per half

    persist = ctx.enter_context(tc.tile_pool(name="persist", bufs=1))
    dram = ctx.enter_context(tc.tile_pool(name="dram", bufs=1, space="DRAM"))

    id_bf = persist.tile([128, 128], BF16)
    make_identity(nc, id_bf)

    g1_sb = persist.tile([128, H_TILES], F32)
    b1_sb = persist.tile([128, H_TILES], F32)
    g2_sb = persist.tile([128, CT_TILES], F32)
    b2_sb = persist.tile([128, CT_TILES], F32)
    nc.sync.dma_start(out=g1_sb, in_=g1.rearrange("(a b) -> b a", b=128))
    nc.sync.dma_start(out=b1_sb, in_=b1.rearrange("(a b) -> b a", b=128))
    nc.sync.dma_start(out=g2_sb, in_=g2.rearrange("(a b) -> b a", b=128))
    nc.sync.dma_start(out=b2_sb, in_=b2.rearrange("(a b) -> b a", b=128))

    h_dram = dram.tile([H_TILES, 128, M], F32)
    spk_dram = dram.tile([M, H], FP8)
    ag1_in = dram.tile([2, 128, 2 * HH], F32)
    ag1_outs = [dram.tile([8, 128, 2 * HH], F32, addr_space="Shared",
                          tag=f"ag1o{i}", name=f"ag1o{i}") for i in range(2)]
    ag2_in = dram.tile([128, 8], F32)
    ag2_out = dram.tile([8, 128, 8], F32, addr_space="Shared",
                        tag="ag2o", name="ag2o")

    # ================= A+B superphase: two ht-halves, interleaved =========
    with tc.tile_pool(name="pA_w", bufs=2) as paw, \
         tc.tile_pool(name="pA_x", bufs=2) as pax, \
         tc.tile_pool(name="pA_h", bufs=4) as pah, \
         tc.tile_pool(name="pA_st", bufs=1) as past, \
         tc.tile_pool(name="pA_ps", bufs=5, space="PSUM") as paps, \
         tc.tile_pool(name="pB_h", bufs=8) as pbh, \
         tc.tile_pool(name="pB_v", bufs=3) as pbv, \
         tc.tile_pool(name="pB_sp", bufs=1) as pbsp, \
         tc.tile_pool(name="pB_stg", bufs=4) as pbstg, \
         tc.tile_pool(name="pB_ps", bufs=2, space="PSUM") as pbps:

        w1sbs = {}
        scales1 = {}
        sp_tiles = {}

        def emit_w1_loads(half, part):
            """part 0: fp32r tiles (needed by the first matmul of each ps
            group); part 1: the bf16 cross-term tiles (consumed later)."""
            csl = slice(half * HCOL, (half + 1) * HCOL)
            if part == 0:
                w1r_sb = []
                for c in range(C_CHUNKS):
                    wr = paw.tile([128, HCOL], F32R, tag=f"w1r{c}",
                                  name=f"w1r{half}_{c}")
                    nc.sync.dma_start(out=wr,
                                      in_=w1Tr[c * 128:(c + 1) * 128, csl])
                    w1r_sb.append(wr)
                w1sbs[half] = (w1r_sb, [], [])
                return
            w1r_sb, w1e_sb, w1h_sb = w1sbs[half]
            for c in range(C_CHUNKS):
                we = paw.tile([128, HCOL], BF16, tag=f"w1e{c}",
                              name=f"w1e{half}_{c}")
                wh = paw.tile([128, HCOL], BF16, tag=f"w1h{c}",
                              name=f"w1h{half}_{c}")
                nc.sync.dma_start(out=we, in_=w1Te[c * 128:(c + 1) * 128, csl])
                nc.sync.dma_start(out=wh, in_=w1Th[c * 128:(c + 1) * 128, csl])
                w1e_sb.append(we)
                w1h_sb.append(wh)

        st_tiles = {}
        x_tiles = {}

        def emit_x_loads(half, mt):
            # load x for two m-tiles at once (halves DMA queue traffic)
            msl2 = slice(mt * M_T, (mt + 2) * M_T)
            xr2, xe2, xh2 = [], [], []
            for c in range(C_CHUNKS):
                xr = pax.tile([128, 2 * M_T], F32R, tag=f"xr{c}",
                              name=f"xr{half}_{mt}_{c}")
                nc.sync.dma_start(out=xr,
                                  in_=xTr[c * 128:(c + 1) * 128, msl2])
                xr2.append(xr)
                xe = pax.tile([128, 2 * M_T], BF16, tag=f"xe{c}",
                              name=f"xe{half}_{mt}_{c}")
                xh = pax.tile([128, 2 * M_T], BF16, tag=f"xh{c}",
                              name=f"xh{half}_{mt}_{c}")
                nc.sync.dma_start(out=xe,
                                  in_=xTe[c * 128:(c + 1) * 128, msl2])
                nc.sync.dma_start(out=xh,
                                  in_=xTh[c * 128:(c + 1) * 128, msl2])
                xe2.append(xe)
                xh2.append(xh)
            x_tiles[half] = (xr2, xe2, xh2)

        def emit_A_mt(half, mt):
            w1r_sb, w1e_sb, w1h_sb = w1sbs[half]
            hts = list(range(half * HH, (half + 1) * HH))
            if mt == 0:
                st_tiles[half] = [past.tile([128, N_MT, 6], F32,
                                            tag=f"st{ht}", name=f"st{ht}")
                                  for ht in hts]
            st = st_tiles[half]
            msl = slice(mt * M_T, (mt + 1) * M_T)
            xr2, xe2, xh2 = x_tiles[half]
            xsl = slice((mt % 2) * M_T, (mt % 2 + 1) * M_T)
            xr_sb = [tt[:, xsl] for tt in xr2]
            xe_sb = [tt[:, xsl] for tt in xe2]
            xh_sb = [tt[:, xsl] for tt in xh2]
            for i_ht, ht in enumerate(hts):
                hsl = slice(i_ht * 128, (i_ht + 1) * 128)
                ps = paps.tile([128, M_T], F32, tag="ps")
                for c in range(C_CHUNKS):
                    nc.tensor.matmul(ps, w1r_sb[c][:, hsl], xr_sb[c],
                                     start=(c == 0), stop=False)
                for c in range(C_CHUNKS):
                    nc.tensor.matmul(ps, w1h_sb[c][:, hsl], xe_sb[c],
                                     start=False, stop=False)
                for c in range(C_CHUNKS):
                    nc.tensor.matmul(ps, w1e_sb[c][:, hsl], xh_sb[c],
                                     start=False, stop=(c == C_CHUNKS - 1))
                nc.vector.bn_stats(out=st[i_ht][:, mt, :], in_=ps)
                hstg = pah.tile([128, M_T], F32, tag="hstg")
                nc.scalar.activation(out=hstg, in_=ps, func=ACTF.Copy)
                nc.sync.dma_start(out=h_dram[ht][:, msl], in_=hstg)

        def emit_stats_ar1(half):
            hts = list(range(half * HH, (half + 1) * HH))
            st = st_tiles[half]
            stats1h = persist.tile([128, 2 * HH], F32, tag=f"stats1h{half}",
                                   name=f"stats1h{half}")
            for i_ht, ht in enumerate(hts):
                mv = past.tile([128, 2], F32, tag="mv", name=f"mv{ht}")
                nc.vector.bn_aggr(out=mv, in_=st[i_ht])
                nc.vector.tensor_copy(stats1h[:, i_ht:i_ht + 1], mv[:, 0:1])
                nc.vector.scalar_tensor_tensor(
                    out=stats1h[:, HH + i_ht:HH + i_ht + 1],
                    in0=mv[:, 0:1], scalar=mv[:, 0:1], in1=mv[:, 1:2],
                    op0=ALU.mult, op1=ALU.add)
            ar1h = _emit_ag_sum(nc, persist, ag1_in[half], ag1_outs[half],
                                stats1h, HH, f"L1h{half}")
            scales1[half] = _emit_stats_to_scales(
                nc, persist, ar1h, g1_sb[:, half * HH:(half + 1) * HH],
                b1_sb[:, half * HH:(half + 1) * HH], HH, f"L1h{half}")

        def emit_B_tile(half, i_ht):
            s1v, c1v = scales1[half]
            ht = half * HH + i_ht
            hbs = []
            for tt in range(T):
                hbt = pbh.tile([128, NN * B_LOC], F32, tag="hb",
                               name=f"hb{ht}_{tt}")
                nc.sync.dma_start(
                    out=hbt,
                    in_=h_dram[ht][:, tt * NN * B_LOC:(tt + 1) * NN * B_LOC])
                hbs.append(hbt)
            sp = pbsp.tile([128, M], BF16, tag=f"sp{i_ht % 4}",
                           name=f"sp{half}_{i_ht}")
            _emit_lif(nc, hbs, sp, pbv,
                      [s[:, i_ht:i_ht + 1] for s in s1v],
                      [cv[:, i_ht:i_ht + 1] for cv in c1v],
                      NN * B_LOC)
            sp_tiles[(half, i_ht)] = sp

        def emit_B_trans(half, hg):
            sps = [sp_tiles[(half, hg * 4 + hh)] for hh in range(4)]
            hcol0 = (half * 2 + hg) * 512
            for mb in range(N_MB):
                pst = pbps.tile([MB, 512], BF16, tag="pst")
                for hh in range(4):
                    nc.tensor.matmul(
                        pst[:, hh * 128:(hh + 1) * 128],
                        sps[hh][:, mb * MB:(mb + 1) * MB],
                        id_bf, is_transpose=True,
                        start=(hh == 0), stop=(hh == 3))
                stg = pbstg.tile([MB, 512], FP8, tag="stg")
                if mb % 3 != 2:
                    nc.scalar.activation(out=stg, in_=pst, func=ACTF.Copy)
                else:
                    nc.vector.tensor_copy(stg, pst)
                nc.sync.dma_start(
                    out=spk_dram[mb * MB:(mb + 1) * MB, hcol0:hcol0 + 512],
                    in_=stg)

        # --- interleaved A/B emission ---
        emit_w1_loads(0, 0)
        emit_x_loads(0, 0)
        emit_w1_loads(0, 1)
        for mt in range(N_MT):
            if mt % 2 == 0 and mt > 0:
                emit_x_loads(0, mt)
            emit_A_mt(0, mt)
        emit_w1_loads(1, 0)
        emit_w1_loads(1, 1)
        emit_stats_ar1(0)
        for mt in range(N_MT):
            if mt % 2 == 0:
                emit_x_loads(1, mt)
            emit_A_mt(1, mt)
            emit_B_tile(0, mt)
            if mt == 3:
                emit_B_trans(0, 0)
        emit_stats_ar1(1)
        emit_B_trans(0, 1)
        for i_ht in range(HH):
            emit_B_tile(1, i_ht)
            if i_ht == 4:
                emit_B_trans(1, 0)
        emit_B_trans(1, 1)

    if stop_after in ('A', 'B'):
        return

    # ================= C+D: fp8 DoubleRow GEMM, all 4 ct per rhs read =====
    with tc.tile_pool(name="pC_w", bufs=1) as pcw, \
         tc.tile_pool(name="pC_r", bufs=3) as pcr, \
         tc.tile_pool(name="pC_st", bufs=1) as pcst, \
         tc.tile_pool(name="pC_ps", bufs=3, space="PSUM") as pcps, \
         tc.tile_pool(name="pO", bufs=1) as po, \
         tc.tile_pool(name="pD_v", bufs=2) as pdv, \
         tc.tile_pool(name="pD_sp", bufs=1) as pdsp, \
         tc.tile_pool(name="pD_stg", bufs=4) as pdstg, \
         tc.tile_pool(name="pD_ps", bufs=2, space="PSUM") as pdps:
        w2sb = [[pcw.tile([128, 2, C], FP8, tag=f"w2_{p}_{icp}",
                          name=f"w2_{p}_{icp}") for icp in range(N_ICP)]
                for p in range(4)]
        for p in range(4):
            for icp in range(N_ICP):
                src = bass.AP(
                    tensor=w2p[p].tensor,
                    offset=w2p[p].offset + icp * 256 * C,
                    ap=[[C, 128], [128 * C, 2], [1, C]])
                nc.sync.dma_start(out=w2sb[p][icp], in_=src)

        o_sb = [po.tile([128, M], F32, tag=f"osb{ct}", name=f"osb{ct}")
                for ct in range(CT_TILES)]
        st2 = [pcst.tile([128, N_MT, 6], F32, tag=f"st2_{ct}",
                         name=f"st2_{ct}") for ct in range(CT_TILES)]
        sp2 = [pdsp.tile([128, M], BF16, tag=f"sp2_{ct}", name=f"sp2_{ct}")
               for ct in range(CT_TILES)]

        def emit_C_mt(mt):
            rhs = []
            for icp in range(N_ICP):
                rt = pcr.tile([128, 2, 2, NN], FP8, tag=f"rhs{icp}",
                              name=f"rhs{mt}_{icp}")
                for r in range(2):
                    src = bass.AP(
                        tensor=spk_dram.tensor,
                        offset=spk_dram.offset + (2 * mt + r) * (NN * H)
                        + icp * 256 * NN,
                        ap=[[NN, 128], [128 * NN, 2], [1, NN]])
                    nc.sync.dma_start(out=rt[:, :, r, :], in_=src)
                rhs.append(rt)
            for ct in range(CT_TILES):
                csl = slice(ct * 128, (ct + 1) * 128)
                psA = pcps.tile([128, M_T], F32, tag="psA")
                psB = pcps.tile([128, M_T], F32, tag="psB")
                for g, ps in ((0, psA), (1, psB)):
                    for pp in range(2):
                        p = 2 * g + pp
                        for icp in range(N_ICP):
                            nc.tensor.matmul(
                                ps, w2sb[p][icp][:, :, csl],
                                rhs[icp][:, :, :, :],
                                perf_mode=DR,
                                start=(pp == 0 and icp == 0),
                                stop=(pp == 1 and icp == N_ICP - 1))
                # rescale to TRUE o magnitude (1/64) so BN's +eps keeps the
                # reference semantics (BN is only scale-invariant if eps
                # scales too); group-1 psum additionally carries 2^-GSH.
                osl = o_sb[ct][:, mt * M_T:(mt + 1) * M_T]
                tmpB = pcr.tile([128, M_T], F32, tag="tmpB")
                nc.scalar.activation(out=tmpB, in_=psB, func=ACTF.Identity,
                                     scale=float(2.0 ** -GSH / 64.0))
                nc.vector.scalar_tensor_tensor(
                    out=osl, in0=psA, scalar=float(1.0 / 64.0), in1=tmpB,
                    op0=ALU.mult, op1=ALU.add)
                nc.vector.bn_stats(out=st2[ct][:, mt, :], in_=osl)

        scales2 = {}

        def emit_stats_ag2():
            stats2 = persist.tile([128, 8], F32, tag="stats2", name="stats2")
            for ct in range(CT_TILES):
                mv2 = pcst.tile([128, 2], F32, tag="mv2", name=f"mv2{ct}")
                nc.vector.bn_aggr(out=mv2, in_=st2[ct])
                nc.vector.tensor_copy(stats2[:, ct:ct + 1], mv2[:, 0:1])
                nc.vector.scalar_tensor_tensor(
                    out=stats2[:, 4 + ct:5 + ct],
                    in0=mv2[:, 0:1], scalar=mv2[:, 0:1], in1=mv2[:, 1:2],
                    op0=ALU.mult, op1=ALU.add)
            ar2_sb = _emit_ag_sum(nc, persist, ag2_in, ag2_out, stats2,
                                  CT_TILES, "L2")
            scales2[0] = _emit_stats_to_scales(
                nc, persist, ar2_sb, g2_sb, b2_sb, CT_TILES, "L2")

        def emit_D_lif(ct):
            s2v, c2v = scales2[0]
            _emit_lif(nc, o_sb[ct], sp2[ct], pdv,
                      [s[:, ct:ct + 1] for s in s2v],
                      [cv[:, ct:ct + 1] for cv in c2v],
                      NN * B_LOC)

        def emit_D_trans(pair):
            cts = [2 * pair, 2 * pair + 1]
            for mb in range(N_MB):
                pst = pdps.tile([MB, 256], BF16, tag="pst2")
                for i_ct, ct in enumerate(cts):
                    nc.tensor.matmul(
                        pst[:, i_ct * 128:(i_ct + 1) * 128],
                        sp2[ct][:, mb * MB:(mb + 1) * MB],
                        id_bf, is_transpose=True,
                        start=(i_ct == 0), stop=(i_ct == 1))
                stg = pdstg.tile([MB, 256], F32, tag="stg2")
                if mb % 2 == 0:
                    nc.scalar.activation(out=stg, in_=pst, func=ACTF.Copy)
                else:
                    nc.vector.tensor_copy(stg, pst)
                nc.sync.dma_start(
                    out=qv[mb * MB:(mb + 1) * MB,
                           pair * 256:(pair + 1) * 256],
                    in_=stg)

        # --- C/D emission ---
        for mt in range(N_MT):
            emit_C_mt(mt)
        emit_stats_ag2()
        emit_D_lif(0)
        emit_D_lif(1)
        emit_D_trans(0)
        emit_D_lif(2)
        emit_D_lif(3)
        emit_D_trans(1)


_NC_CACHE = None
LAST_RES = None


def _get_nc():
    global _NC_CACHE
    if _NC_CACHE is None:
        _NC_CACHE = build_program()
    return _NC_CACHE


def kernel(x, w1, g1, b1, w2, g2, b2):
    x = np.asarray(x, dtype=np.float32)
    w1 = np.asarray(w1, dtype=np.float32)
    w2 = np.asarray(w2, dtype=np.float32)
    g1 = np.asarray(g1, dtype=np.float32)
    b1 = np.asarray(b1, dtype=np.float32)
    g2 = np.asarray(g2, dtype=np.float32)
    b2 = np.asarray(b2, dtype=np.float32)

    w1T = np.ascontiguousarray(w1.T)                    # (C, H)
    w2T = np.ascontiguousarray(w2.T).astype(np.float32)  # (H, C)
    # fp8 scale-grouped split: two groups of two e4m3 parts. Group 0 at
    # scale 2^6, group 1 at 2^(6+GSH); BN cancels the global 2^6, the
    # kernel applies 2^-GSH when combining the two group psums.
    e4 = ml_dtypes.float8_e4m3
    w2s = (w2T * 64.0).astype(np.float32)
    p0 = w2s.astype(e4)
    r1 = w2s - p0.astype(np.float32)
    p1 = r1.astype(e4)
    r2 = ((w2s - p0.astype(np.float32) - p1.astype(np.float32))
          * float(2.0 ** GSH)).astype(np.float32)
    p2 = r2.astype(e4)
    p3 = (r2 - p2.astype(np.float32)).astype(e4)
    w1Tr = rne_keep(w1T, 11)
    w1Te = (w1T - w1Tr).astype(ml_dtypes.bfloat16)
    w1Th = w1T.astype(ml_dtypes.bfloat16)

    xr = x.reshape(T, B_GLOB, NN, C)
    in_maps = []
    for k in range(N_CORES):
        xk = xr[:, k * B_LOC:(k + 1) * B_LOC].reshape(M, C)
        xTk = np.ascontiguousarray(xk.T)                # (C, M)
        xTr = rne_keep(xTk, 11)
        in_maps.append({
            "xTr": xTr,
            "xTe": (xTk - xTr).astype(ml_dtypes.bfloat16),
            "xTh": xTk.astype(ml_dtypes.bfloat16),
            "w1Tr": w1Tr, "w1Te": w1Te, "w1Th": w1Th,
            "w2p0": p0, "w2p1": p1, "w2p2": p2, "w2p3": p3,
            "g1": g1, "b1": b1, "g2": g2, "b2": b2,
            "chain": np.zeros((1, 128), np.float32),
        })

    nc = _get_nc()
    import os
    trace = bool(int(os.environ.get("KERNEL_TRACE", "0")))
    res = run_bass_kernel_spmd(nc, in_maps, core_ids=list(range(N_CORES)),
                               trace=trace)
    global LAST_RES
    LAST_RES = res

    out = np.empty((T * B_GLOB, NN, C), dtype=np.float32)
    outr = out.reshape(T, B_GLOB, NN, C)
    for k in range(N_CORES):
        qvk = res.results[k]["qv"]                      # (M, C) in (r, n, c)
        tmp = qvk.reshape(R, C, NN).transpose(0, 2, 1)  # (R, N, C)
        outr[:, k * B_LOC:(k + 1) * B_LOC] = tmp.reshape(T, B_LOC, NN, C)
    return out


# revision 12
# speedup vs baseline: 1.1872x; 1.0027x over previous
"""Trainium2 Bass kernel for nn_Decoder_MLP: Linear->BN->LIF -> Linear->BN->LIF.

Sharding: data-parallel over batch B (TB=T*B=128 rows -> 4 batch items/core,
all T=4 timesteps local). BN batch stats are all-gathered (cheaper than
all-reduce in the collective model) and summed locally on DVE.

Reference semantics replicated exactly, including the "scrambled" reshapes
(T,B,N,H)->(TB,H,N) which reinterpret (N,H) blocks as (H,N) row-major. That
scramble is handled by writing LIF-1 spikes to DRAM in (m, h) row-major order
(m = r*196+n) and reading them back with a strided access pattern as
(i, m') tiles, where i = the scrambled contraction index.

Layer-1 matmul runs fp32r (RNE-11) + two bf16 cross-term passes (~fp32
quality). Layer-2 runs on exact fp8(e4m3) spikes with w2 split into two
scale-groups of two e4m3 parts each, using DoubleRow fp8 matmuls (0.5
cyc/row); the group scales cancel in BN (scale-invariant), the inter-group
2^-8 factor is applied in the psum-combine step.
"""

import numpy as np
import ml_dtypes

import concourse.bass as bass
import concourse.mybir as mybir
import concourse.tile as tile
from concourse import bacc
from concourse.bass_utils import run_bass_kernel_spmd
from concourse.masks import make_identity

F32 = mybir.dt.float32
F32R = mybir.dt.float32r
BF16 = mybir.dt.bfloat16
FP8 = mybir.dt.float8e4
ALU = mybir.AluOpType
ACTF = mybir.ActivationFunctionType
DR = mybir.MatmulPerfMode.DoubleRow

N_CORES = 8
T = 4
B_GLOB = 32
B_LOC = B_GLOB // N_CORES          # 4 batch items per core
R = T * B_LOC                      # 16 local (t, b) rows
NN = 196                           # sequence/pixels dim N
C = 512
H = 2048
M = R * NN                         # 3136 local rows of the flattened GEMM
M_T = 392                          # = 2*NN, keeps m-tiles r-aligned
N_MT = M // M_T                    # 8
MB = 112                           # transpose block (3136 = 28*112)
N_MB = M // MB                     # 28
C_CHUNKS = C // 128                # 4
H_TILES = H // 128                 # 16
CT_TILES = C // 128                # 4
N_ICP = H // 256                   # 8 DoubleRow contraction chunks
EPS = 1e-5
GSH = 8                            # fp8 scale-group shift (2^GSH)


def rne_keep(x, bits):
    """Round-to-nearest-even keeping `bits` explicit fp32 mantissa bits —
    bit-exact emulation of the PE's fp32r input rounding (measured RNE-11)."""
    u = np.ascontiguousarray(x, dtype=np.float32).view(np.uint32)
    shift = 23 - bits
    half = np.uint32(1 << (shift - 1))
    lsb = np.uint32(1 << shift)
    mask = np.uint32(~(lsb - np.uint32(1)))
    out = (u + half - np.uint32(1) + ((u >> np.uint32(shift)) & np.uint32(1))) & mask
    return out.view(np.float32)


def _emit_lif(nc, sb_h, sp_out, vpool, s_vecs, c_vecs, n_free, n_t=T):
    """BN-apply + LIF on a (channel 128, m) tile; m = (t, b, n) t-major.
    V_t = V'_{t-1} + 2^{t-1}*(scale*h_t + shift); spike iff V_t >= 2^t;
    V'_t = V_t * (V_t < 2^t)."""
    vprev = None
    for t in range(n_t):
        thr = float(2.0 ** (t + 1))
        hsl = sb_h[t] if isinstance(sb_h, list) else \
            sb_h[:, t * n_free:(t + 1) * n_free]
        if t == 0:
            v = vpool.tile([128, n_free], F32, tag="v")
            nc.scalar.activation(out=v, in_=hsl, func=ACTF.Identity,
                                 bias=c_vecs[t], scale=s_vecs[t])
        else:
            y = vpool.tile([128, n_free], F32, tag="y")
            nc.scalar.activation(out=y, in_=hsl, func=ACTF.Identity,
                                 bias=c_vecs[t], scale=s_vecs[t])
            v = vpool.tile([128, n_free], F32, tag="v")
            nc.vector.tensor_tensor(out=v, in0=vprev, in1=y, op=ALU.add)
        nc.gpsimd.tensor_scalar(out=sp_out[:, t * n_free:(t + 1) * n_free],
                                in0=v, scalar1=thr, scalar2=None, op0=ALU.is_ge)
        if t < n_t - 1:
            vp = vpool.tile([128, n_free], F32, tag="vp")
            nc.vector.scalar_tensor_tensor(out=vp, in0=v, scalar=thr, in1=v,
                                           op0=ALU.is_lt, op1=ALU.mult)
            vprev = vp


def _emit_ag_sum(nc, pool, ag_in, ag_out, stats_sb, w, sfx):
    """AllGather per-core stats ([128, 2w]) then 8-way local sum on DVE.
    Returns the summed [128, 2w] SBUF tile."""
    nc.sync.dma_start(out=ag_in, in_=stats_sb)
    nc.gpsimd.collective_compute(
        "AllGather", ALU.bypass, replica_groups=[list(range(N_CORES))],
        ins=[ag_in.opt()], outs=[ag_out.opt()])
    agg = pool.tile([128, 8, 2 * w], F32, tag=f"agg{sfx}", name=f"agg{sfx}")
    nc.sync.dma_start(out=agg, in_=ag_out.rearrange("a p w -> p a w"))
    t4 = pool.tile([128, 4, 2 * w], F32, tag=f"t4{sfx}", name=f"t4{sfx}")
    nc.vector.tensor_tensor(out=t4, in0=agg[:, 0:4], in1=agg[:, 4:8],
                            op=ALU.add)
    t2 = pool.tile([128, 2, 2 * w], F32, tag=f"t2{sfx}", name=f"t2{sfx}")
    nc.vector.tensor_tensor(out=t2, in0=t4[:, 0:2], in1=t4[:, 2:4],
                            op=ALU.add)
    t1 = pool.tile([128, 2 * w], F32, tag=f"t1{sfx}", name=f"t1{sfx}")
    nc.vector.tensor_tensor(out=t1, in0=t2[:, 0:1], in1=t2[:, 1:2],
                            op=ALU.add)
    return t1


def _emit_stats_to_scales(nc, pool, ar_sb, gamma_sb, beta_sb, w, sfx):
    """ar_sb: (128, 2w) summed [sum-of-means | sum-of-Ex2]. Returns per-t
    (s_vecs, c_vecs) lists of (128, w) tiles: 2^t*scale, 2^t*shift."""
    mean = pool.tile([128, w], F32, tag=f"bnmean{sfx}", name=f"bnmean{sfx}")
    ex2 = pool.tile([128, w], F32, tag=f"bnex2{sfx}", name=f"bnex2{sfx}")
    nc.vector.tensor_scalar(out=mean, in0=ar_sb[:, 0:w], scalar1=1.0 / N_CORES,
                            scalar2=None, op0=ALU.mult)
    nc.vector.tensor_scalar(out=ex2, in0=ar_sb[:, w:2 * w],
                            scalar1=1.0 / N_CORES, scalar2=None, op0=ALU.mult)
    var = pool.tile([128, w], F32, tag=f"bnvar{sfx}", name=f"bnvar{sfx}")
    msq = pool.tile([128, w], F32, tag=f"bnmsq{sfx}", name=f"bnmsq{sfx}")
    nc.vector.tensor_tensor(out=msq, in0=mean, in1=mean, op=ALU.mult)
    nc.vector.tensor_tensor(out=var, in0=ex2, in1=msq, op=ALU.subtract)
    epsb = pool.tile([128, 1], F32, tag=f"bneps{sfx}", name=f"bneps{sfx}")
    nc.vector.memset(epsb, EPS)
    std = pool.tile([128, w], F32, tag=f"bnstd{sfx}", name=f"bnstd{sfx}")
    nc.scalar.activation(out=std, in_=var, func=ACTF.Sqrt, bias=epsb, scale=1.0)
    rstd = pool.tile([128, w], F32, tag=f"bnrstd{sfx}", name=f"bnrstd{sfx}")
    nc.vector.reciprocal(out=rstd, in_=std)
    scale = pool.tile([128, w], F32, tag=f"bnscale{sfx}", name=f"bnscale{sfx}")
    nc.vector.tensor_tensor(out=scale, in0=gamma_sb, in1=rstd, op=ALU.mult)
    mscl = pool.tile([128, w], F32, tag=f"bnmscl{sfx}", name=f"bnmscl{sfx}")
    nc.vector.tensor_tensor(out=mscl, in0=mean, in1=scale, op=ALU.mult)
    shift = pool.tile([128, w], F32, tag=f"bnshift{sfx}", name=f"bnshift{sfx}")
    nc.vector.tensor_tensor(out=shift, in0=beta_sb, in1=mscl, op=ALU.subtract)
    s_vecs, c_vecs = [], []
    for t in range(T):
        f = float(2.0 ** t)
        s = pool.tile([128, w], F32, tag=f"bns{t}{sfx}", name=f"bns{t}{sfx}")
        cc = pool.tile([128, w], F32, tag=f"bnc{t}{sfx}", name=f"bnc{t}{sfx}")
        nc.vector.tensor_scalar(out=s, in0=scale, scalar1=f, scalar2=None,
                                op0=ALU.mult)
        nc.vector.tensor_scalar(out=cc, in0=shift, scalar1=f, scalar2=None,
                                op0=ALU.mult)
        s_vecs.append(s)
        c_vecs.append(cc)
    return s_vecs, c_vecs


def build_program(stop_after='D'):
    nc = bacc.Bacc("TRN2", target_bir_lowering=False, debug=False,
                   num_devices=N_CORES)

    xTr = nc.dram_tensor("xTr", [C, M], F32R, kind="ExternalInput").ap()
    xTe = nc.dram_tensor("xTe", [C, M], BF16, kind="ExternalInput").ap()
    xTh = nc.dram_tensor("xTh", [C, M], BF16, kind="ExternalInput").ap()
    w1Tr = nc.dram_tensor("w1Tr", [C, H], F32R, kind="ExternalInput").ap()
    w1Te = nc.dram_tensor("w1Te", [C, H], BF16, kind="ExternalInput").ap()
    w1Th = nc.dram_tensor("w1Th", [C, H], BF16, kind="ExternalInput").ap()
    w2p = [nc.dram_tensor(f"w2p{p}", [H, C], FP8, kind="ExternalInput").ap()
           for p in range(4)]
    g1 = nc.dram_tensor("g1", [H], F32, kind="ExternalInput").ap()
    b1 = nc.dram_tensor("b1", [H], F32, kind="ExternalInput").ap()
    g2 = nc.dram_tensor("g2", [C], F32, kind="ExternalInput").ap()
    b2 = nc.dram_tensor("b2", [C], F32, kind="ExternalInput").ap()
    qv = nc.dram_tensor("qv", [M, C], F32, kind="ExternalOutput").ap()
    chain = nc.dram_tensor("chain", [1, 128], F32, kind="ExternalInput").ap()
    chain_o = nc.dram_tensor("chain_o", [1, 128], F32, kind="ExternalOutput").ap()

    with tile.TileContext(nc) as tc:
        from contextlib import ExitStack
        with ExitStack() as ctx:
            _build_body(nc, tc, ctx, (xTr, xTe, xTh), (w1Tr, w1Te, w1Th),
                        w2p, g1, b1, g2, b2, qv, stop_after)
        with tc.tile_pool(name="chainp", bufs=1) as chp:
            cht = chp.tile([1, 128], F32)
            nc.sync.dma_start(out=cht, in_=chain)
            nc.sync.dma_start(out=chain_o, in_=cht)
    nc.compile()
    return nc


def _build_body(nc, tc, ctx, x_in, w1_in, w2p, g1, b1, g2, b2, qv,
                stop_after='D'):
    xTr, xTe, xTh = x_in
    w1Tr, w1Te, w1Th = w1_in
    HH = H_TILES // 2            # 8 h-tiles per half
    HCOL = HH * 128              # 1024 h columns per half

    persist = ctx.enter_context(tc.tile_pool(name="persist", bufs=1))
    dram = ctx.enter_context(tc.tile_pool(name="dram", bufs=1, space="DRAM"))

    id_bf = persist.tile([128, 128], BF16)
    make_identity(nc, id_bf)

    g1_sb = persist.tile([128, H_TILES], F32)
    b1_sb = persist.tile([128, H_TILES], F32)
    g2_sb = persist.tile([128, CT_TILES], F32)
    b2_sb = persist.tile([128, CT_TILES], F32)
    nc.sync.dma_start(out=g1_sb, in_=g1.rearrange("(a b) -> b a", b=128))
    nc.sync.dma_start(out=b1_sb, in_=b1.rearrange("(a b) -> b a", b=128))
    nc.sync.dma_start(out=g2_sb, in_=g2.rearrange("(a b) -> b a", b=128))
    nc.sync.dma_start(out=b2_sb, in_=b2.rearrange("(a b) -> b a", b=128))

    h_dram = dram.tile([H_TILES, 128, M], F32)
    spk_dram = dram.tile([M, H], FP8)
    ag1_in = dram.tile([2, 128, 2 * HH], F32)
    ag1_outs = [dram.tile([8, 128, 2 * HH], F32, addr_space="Shared",
                          tag=f"ag1o{i}", name=f"ag1o{i}") for i in range(2)]
    ag2_in = dram.tile([128, 8], F32)
    ag2_out = dram.tile([8, 128, 8], F32, addr_space="Shared",
                        tag="ag2o", name="ag2o")

    # ================= A+B superphase: two ht-halves, interleaved =========
    with tc.tile_pool(name="pA_w", bufs=2) as paw, \
         tc.tile_pool(name="pA_x", bufs=2) as pax, \
         tc.tile_pool(name="pA_h", bufs=4) as pah, \
         tc.tile_pool(name="pA_st", bufs=1) as past, \
         tc.tile_pool(name="pA_ps", bufs=6, space="PSUM") as paps, \
         tc.tile_pool(name="pB_h", bufs=8) as pbh, \
         tc.tile_pool(name="pB_v", bufs=3) as pbv, \
         tc.tile_pool(name="pB_sp", bufs=1) as pbsp, \
         tc.tile_pool(name="pB_stg", bufs=4) as pbstg, \
         tc.tile_pool(name="pB_ps", bufs=2, space="PSUM") as pbps:

        w1sbs = {}
        scales1 = {}
        sp_tiles = {}

        def emit_w1_loads(half, part):
            """part 0: fp32r tiles (needed by the first matmul of each ps
            group); part 1: the bf16 cross-term tiles (consumed later)."""
            csl = slice(half * HCOL, (half + 1) * HCOL)
            if part == 0:
                w1r_sb = []
                for c in range(C_CHUNKS):
                    wr = paw.tile([128, HCOL], F32R, tag=f"w1r{c}",
                                  name=f"w1r{half}_{c}")
                    nc.sync.dma_start(out=wr,
                                      in_=w1Tr[c * 128:(c + 1) * 128, csl])
                    w1r_sb.append(wr)
                w1sbs[half] = (w1r_sb, [], [])
                return
            w1r_sb, w1e_sb, w1h_sb = w1sbs[half]
            for c in range(C_CHUNKS):
                we = paw.tile([128, HCOL], BF16, tag=f"w1e{c}",
                              name=f"w1e{half}_{c}")
                wh = paw.tile([128, HCOL], BF16, tag=f"w1h{c}",
                              name=f"w1h{half}_{c}")
                nc.sync.dma_start(out=we, in_=w1Te[c * 128:(c + 1) * 128, csl])
                nc.sync.dma_start(out=wh, in_=w1Th[c * 128:(c + 1) * 128, csl])
                w1e_sb.append(we)
                w1h_sb.append(wh)

        st_tiles = {}
        x_tiles = {}

        def emit_x_loads(half, mt):
            # load x for two m-tiles at once (halves DMA queue traffic)
            msl2 = slice(mt * M_T, (mt + 2) * M_T)
            xr2, xe2, xh2 = [], [], []
            for c in range(C_CHUNKS):
                xr = pax.tile([128, 2 * M_T], F32R, tag=f"xr{c}",
                              name=f"xr{half}_{mt}_{c}")
                nc.sync.dma_start(out=xr,
                                  in_=xTr[c * 128:(c + 1) * 128, msl2])
                xr2.append(xr)
                xe = pax.tile([128, 2 * M_T], BF16, tag=f"xe{c}",
                              name=f"xe{half}_{mt}_{c}")
                xh = pax.tile([128, 2 * M_T], BF16, tag=f"xh{c}",
                              name=f"xh{half}_{mt}_{c}")
                nc.sync.dma_start(out=xe,
                                  in_=xTe[c * 128:(c + 1) * 128, msl2])
                nc.sync.dma_start(out=xh,
                                  in_=xTh[c * 128:(c + 1) * 128, msl2])
                xe2.append(xe)
                xh2.append(xh)
            x_tiles[half] = (xr2, xe2, xh2)

        def emit_A_mt(half, mt):
            w1r_sb, w1e_sb, w1h_sb = w1sbs[half]
            hts = list(range(half * HH, (half + 1) * HH))
            if mt == 0:
                st_tiles[half] = [past.tile([128, N_MT, 6], F32,
                                            tag=f"st{ht}", name=f"st{ht}")
                                  for ht in hts]
            st = st_tiles[half]
            msl = slice(mt * M_T, (mt + 1) * M_T)
            xr2, xe2, xh2 = x_tiles[half]
            xsl = slice((mt % 2) * M_T, (mt % 2 + 1) * M_T)
            xr_sb = [tt[:, xsl] for tt in xr2]
            xe_sb = [tt[:, xsl] for tt in xe2]
            xh_sb = [tt[:, xsl] for tt in xh2]
            for i_ht, ht in enumerate(hts):
                hsl = slice(i_ht * 128, (i_ht + 1) * 128)
                ps = paps.tile([128, M_T], F32, tag="ps")
                for c in range(C_CHUNKS):
                    nc.tensor.matmul(ps, w1r_sb[c][:, hsl], xr_sb[c],
                                     start=(c == 0), stop=False)
                for c in range(C_CHUNKS):
                    nc.tensor.matmul(ps, w1h_sb[c][:, hsl], xe_sb[c],
                                     start=False, stop=False)
                for c in range(C_CHUNKS):
                    nc.tensor.matmul(ps, w1e_sb[c][:, hsl], xh_sb[c],
                                     start=False, stop=(c == C_CHUNKS - 1))
                nc.vector.bn_stats(out=st[i_ht][:, mt, :], in_=ps)
                hstg = pah.tile([128, M_T], F32, tag="hstg")
                nc.scalar.activation(out=hstg, in_=ps, func=ACTF.Copy)
                nc.sync.dma_start(out=h_dram[ht][:, msl], in_=hstg)

        def emit_stats_ar1(half):
            hts = list(range(half * HH, (half + 1) * HH))
            st = st_tiles[half]
            stats1h = persist.tile([128, 2 * HH], F32, tag=f"stats1h{half}",
                                   name=f"stats1h{half}")
            for i_ht, ht in enumerate(hts):
                mv = past.tile([128, 2], F32, tag="mv", name=f"mv{ht}")
                nc.vector.bn_aggr(out=mv, in_=st[i_ht])
                nc.vector.tensor_copy(stats1h[:, i_ht:i_ht + 1], mv[:, 0:1])
                nc.vector.scalar_tensor_tensor(
                    out=stats1h[:, HH + i_ht:HH + i_ht + 1],
                    in0=mv[:, 0:1], scalar=mv[:, 0:1], in1=mv[:, 1:2],
                    op0=ALU.mult, op1=ALU.add)
            ar1h = _emit_ag_sum(nc, persist, ag1_in[half], ag1_outs[half],
                                stats1h, HH, f"L1h{half}")
            scales1[half] = _emit_stats_to_scales(
                nc, persist, ar1h, g1_sb[:, half * HH:(half + 1) * HH],
                b1_sb[:, half * HH:(half + 1) * HH], HH, f"L1h{half}")

        def emit_B_tile(half, i_ht):
            s1v, c1v = scales1[half]
            ht = half * HH + i_ht
            hbs = []
            for tt in range(T):
                hbt = pbh.tile([128, NN * B_LOC], F32, tag="hb",
                               name=f"hb{ht}_{tt}")
                nc.sync.dma_start(
                    out=hbt,
                    in_=h_dram[ht][:, tt * NN * B_LOC:(tt + 1) * NN * B_LOC])
                hbs.append(hbt)
            sp = pbsp.tile([128, M], BF16, tag=f"sp{i_ht % 4}",
                           name=f"sp{half}_{i_ht}")
            _emit_lif(nc, hbs, sp, pbv,
                      [s[:, i_ht:i_ht + 1] for s in s1v],
                      [cv[:, i_ht:i_ht + 1] for cv in c1v],
                      NN * B_LOC)
            sp_tiles[(half, i_ht)] = sp

        def emit_B_trans(half, hg):
            sps = [sp_tiles[(half, hg * 4 + hh)] for hh in range(4)]
            hcol0 = (half * 2 + hg) * 512
            for mb in range(N_MB):
                pst = pbps.tile([MB, 512], BF16, tag="pst")
                for hh in range(4):
                    nc.tensor.matmul(
                        pst[:, hh * 128:(hh + 1) * 128],
                        sps[hh][:, mb * MB:(mb + 1) * MB],
                        id_bf, is_transpose=True,
                        start=(hh == 0), stop=(hh == 3))
                stg = pbstg.tile([MB, 512], FP8, tag="stg")
                if mb % 3 != 2:
                    nc.scalar.activation(out=stg, in_=pst, func=ACTF.Copy)
                else:
                    nc.vector.tensor_copy(stg, pst)
                nc.sync.dma_start(
                    out=spk_dram[mb * MB:(mb + 1) * MB, hcol0:hcol0 + 512],
                    in_=stg)

        # --- interleaved A/B emission ---
        emit_w1_loads(0, 0)
        emit_x_loads(0, 0)
        emit_w1_loads(0, 1)
        for mt in range(N_MT):
            if mt % 2 == 0 and mt > 0:
                emit_x_loads(0, mt)
            emit_A_mt(0, mt)
        emit_w1_loads(1, 0)
        emit_w1_loads(1, 1)
        emit_stats_ar1(0)
        for mt in range(N_MT):
            if mt % 2 == 0:
                emit_x_loads(1, mt)
            emit_A_mt(1, mt)
            emit_B_tile(0, mt)
            if mt == 3:
                emit_B_trans(0, 0)
        emit_stats_ar1(1)
        emit_B_trans(0, 1)
        for i_ht in range(HH):
            emit_B_tile(1, i_ht)
            if i_ht == 4:
                emit_B_trans(1, 0)
        emit_B_trans(1, 1)

    if stop_after in ('A', 'B'):
        return

    # ================= C+D: fp8 DoubleRow GEMM, all 4 ct per rhs read =====
    with tc.tile_pool(name="pC_w", bufs=1) as pcw, \
         tc.tile_pool(name="pC_r", bufs=4) as pcr, \
         tc.tile_pool(name="pC_st", bufs=1) as pcst, \
         tc.tile_pool(name="pC_ps", bufs=3, space="PSUM") as pcps, \
         tc.tile_pool(name="pO", bufs=1) as po, \
         tc.tile_pool(name="pD_v", bufs=2) as pdv, \
         tc.tile_pool(name="pD_sp", bufs=1) as pdsp, \
         tc.tile_pool(name="pD_stg", bufs=4) as pdstg, \
         tc.tile_pool(name="pD_ps", bufs=2, space="PSUM") as pdps:
        w2sb = [[pcw.tile([128, 2, C], FP8, tag=f"w2_{p}_{icp}",
                          name=f"w2_{p}_{icp}") for icp in range(N_ICP)]
                for p in range(4)]
        for p in range(4):
            for icp in range(N_ICP):
                src = bass.AP(
                    tensor=w2p[p].tensor,
                    offset=w2p[p].offset + icp * 256 * C,
                    ap=[[C, 128], [128 * C, 2], [1, C]])
                nc.sync.dma_start(out=w2sb[p][icp], in_=src)

        o_sb = [po.tile([128, M], F32, tag=f"osb{ct}", name=f"osb{ct}")
                for ct in range(CT_TILES)]
        st2 = [pcst.tile([128, N_MT, 6], F32, tag=f"st2_{ct}",
                         name=f"st2_{ct}") for ct in range(CT_TILES)]
        sp2 = [pdsp.tile([128, M], BF16, tag=f"sp2_{ct}", name=f"sp2_{ct}")
               for ct in range(CT_TILES)]

        def emit_C_mt(mt):
            rhs = []
            for icp in range(N_ICP):
                rt = pcr.tile([128, 2, 2, NN], FP8, tag=f"rhs{icp}",
                              name=f"rhs{mt}_{icp}")
                for r in range(2):
                    src = bass.AP(
                        tensor=spk_dram.tensor,
                        offset=spk_dram.offset + (2 * mt + r) * (NN * H)
                        + icp * 256 * NN,
                        ap=[[NN, 128], [128 * NN, 2], [1, NN]])
                    nc.sync.dma_start(out=rt[:, :, r, :], in_=src)
                rhs.append(rt)
            for ct in range(CT_TILES):
                csl = slice(ct * 128, (ct + 1) * 128)
                psA = pcps.tile([128, M_T], F32, tag="psA")
                psB = pcps.tile([128, M_T], F32, tag="psB")
                for g, ps in ((0, psA), (1, psB)):
                    for pp in range(2):
                        p = 2 * g + pp
                        for icp in range(N_ICP):
                            nc.tensor.matmul(
                                ps, w2sb[p][icp][:, :, csl],
                                rhs[icp][:, :, :, :],
                                perf_mode=DR,
                                start=(pp == 0 and icp == 0),
                                stop=(pp == 1 and icp == N_ICP - 1))
                # rescale to TRUE o magnitude (1/64) so BN's +eps keeps the
                # reference semantics (BN is only scale-invariant if eps
                # scales too); group-1 psum additionally carries 2^-GSH.
                osl = o_sb[ct][:, mt * M_T:(mt + 1) * M_T]
                tmpB = pcr.tile([128, M_T], F32, tag="tmpB")
                nc.scalar.activation(out=tmpB, in_=psB, func=ACTF.Identity,
                                     scale=float(2.0 ** -GSH / 64.0))
                nc.vector.scalar_tensor_tensor(
                    out=osl, in0=psA, scalar=float(1.0 / 64.0), in1=tmpB,
                    op0=ALU.mult, op1=ALU.add)
                nc.vector.bn_stats(out=st2[ct][:, mt, :], in_=osl)

        scales2 = {}

        def emit_stats_ag2():
            stats2 = persist.tile([128, 8], F32, tag="stats2", name="stats2")
            for ct in range(CT_TILES):
                mv2 = pcst.tile([128, 2], F32, tag="mv2", name=f"mv2{ct}")
                nc.vector.bn_aggr(out=mv2, in_=st2[ct])
                nc.vector.tensor_copy(stats2[:, ct:ct + 1], mv2[:, 0:1])
                nc.vector.scalar_tensor_tensor(
                    out=stats2[:, 4 + ct:5 + ct],
                    in0=mv2[:, 0:1], scalar=mv2[:, 0:1], in1=mv2[:, 1:2],
                    op0=ALU.mult, op1=ALU.add)
            ar2_sb = _emit_ag_sum(nc, persist, ag2_in, ag2_out, stats2,
                                  CT_TILES, "L2")
            scales2[0] = _emit_stats_to_scales(
                nc, persist, ar2_sb, g2_sb, b2_sb, CT_TILES, "L2")

        def emit_D_lif(ct):
            s2v, c2v = scales2[0]
            _emit_lif(nc, o_sb[ct], sp2[ct], pdv,
                      [s[:, ct:ct + 1] for s in s2v],
                      [cv[:, ct:ct + 1] for cv in c2v],
                      NN * B_LOC)

        def emit_D_trans(pair):
            cts = [2 * pair, 2 * pair + 1]
            for mb in range(N_MB):
                pst = pdps.tile([MB, 256], BF16, tag="pst2")
                for i_ct, ct in enumerate(cts):
                    nc.tensor.matmul(
                        pst[:, i_ct * 128:(i_ct + 1) * 128],
                        sp2[ct][:, mb * MB:(mb + 1) * MB],
                        id_bf, is_transpose=True,
                        start=(i_ct == 0), stop=(i_ct == 1))
                stg = pdstg.tile([MB, 256], F32, tag="stg2")
                if mb % 2 == 0:
                    nc.scalar.activation(out=stg, in_=pst, func=ACTF.Copy)
                else:
                    nc.vector.tensor_copy(stg, pst)
                nc.sync.dma_start(
                    out=qv[mb * MB:(mb + 1) * MB,
                           pair * 256:(pair + 1) * 256],
                    in_=stg)

        # --- C/D emission ---
        for mt in range(N_MT):
            emit_C_mt(mt)
        emit_stats_ag2()
        emit_D_lif(0)
        emit_D_lif(1)
        emit_D_trans(0)
        emit_D_lif(2)
        emit_D_lif(3)
        emit_D_trans(1)


_NC_CACHE = None
LAST_RES = None


def _get_nc():
    global _NC_CACHE
    if _NC_CACHE is None:
        _NC_CACHE = build_program()
    return _NC_CACHE


def kernel(x, w1, g1, b1, w2, g2, b2):
    x = np.asarray(x, dtype=np.float32)
    w1 = np.asarray(w1, dtype=np.float32)
    w2 = np.asarray(w2, dtype=np.float32)
    g1 = np.asarray(g1, dtype=np.float32)
    b1 = np.asarray(b1, dtype=np.float32)
    g2 = np.asarray(g2, dtype=np.float32)
    b2 = np.asarray(b2, dtype=np.float32)

    w1T = np.ascontiguousarray(w1.T)                    # (C, H)
    w2T = np.ascontiguousarray(w2.T).astype(np.float32)  # (H, C)
    # fp8 scale-grouped split: two groups of two e4m3 parts. Group 0 at
    # scale 2^6, group 1 at 2^(6+GSH); BN cancels the global 2^6, the
    # kernel applies 2^-GSH when combining the two group psums.
    e4 = ml_dtypes.float8_e4m3
    w2s = (w2T * 64.0).astype(np.float32)
    p0 = w2s.astype(e4)
    r1 = w2s - p0.astype(np.float32)
    p1 = r1.astype(e4)
    r2 = ((w2s - p0.astype(np.float32) - p1.astype(np.float32))
          * float(2.0 ** GSH)).astype(np.float32)
    p2 = r2.astype(e4)
    p3 = (r2 - p2.astype(np.float32)).astype(e4)
    w1Tr = rne_keep(w1T, 11)
    w1Te = (w1T - w1Tr).astype(ml_dtypes.bfloat16)
    w1Th = w1T.astype(ml_dtypes.bfloat16)

    xr = x.reshape(T, B_GLOB, NN, C)
    in_maps = []
    for k in range(N_CORES):
        xk = xr[:, k * B_LOC:(k + 1) * B_LOC].reshape(M, C)
        xTk = np.ascontiguousarray(xk.T)                # (C, M)
        xTr = rne_keep(xTk, 11)
        in_maps.append({
            "xTr": xTr,
            "xTe": (xTk - xTr).astype(ml_dtypes.bfloat16),
            "xTh": xTk.astype(ml_dtypes.bfloat16),
            "w1Tr": w1Tr, "w1Te": w1Te, "w1Th": w1Th,
            "w2p0": p0, "w2p1": p1, "w2p2": p2, "w2p3": p3,
            "g1": g1, "b1": b1, "g2": g2, "b2": b2,
            "chain": np.zeros((1, 128), np.float32),
        })

    nc = _get_nc()
    import os
    trace = bool(int(os.environ.get("KERNEL_TRACE", "0")))
    res = run_bass_kernel_spmd(nc, in_maps, core_ids=list(range(N_CORES)),
                               trace=trace)
    global LAST_RES
    LAST_RES = res

    out = np.empty((T * B_GLOB, NN, C), dtype=np.float32)
    outr = out.reshape(T, B_GLOB, NN, C)
    for k in range(N_CORES):
        qvk = res.results[k]["qv"]                      # (M, C) in (r, n, c)
        tmp = qvk.reshape(R, C, NN).transpose(0, 2, 1)  # (R, N, C)
        outr[:, k * B_LOC:(k + 1) * B_LOC] = tmp.reshape(T, B_LOC, NN, C)
    return out
